# revision 11
# baseline (speedup 1.0000x reference)
"""Distributed GQA attention kernel for 8 TRN2 NeuronCores.

Problem: B=2, S=2048, D=2048, 32 q-heads / 8 kv-heads, hd=64, causal + RoPE.

Strategy (kv-head tensor parallel, zero collectives):
  - Core c owns kv-head c (q-heads 4c..4c+3) for BOTH batches over ALL rows.
    Every core loads the full x (host-pretransposed to xT bf16) and projects
    Q (4 heads), K, V (1 kv head each) for all 4096 rows. K/V never leave the
    core, so there are NO collectives. Each core computes a PARTIAL output
    (its 4 heads x its 256 wo rows) and the host sums the 8 partials.
  - Because every core sees all rows, the causal structure is IDENTICAL on
    all cores (SPMD-compatible): per q-block qb only key blocks kb <= qb are
    computed -> ~47% of score/exp/PV work skipped exactly, with all matmuls
    staying N=512 wide (4 q-heads x 128 rows share one kv head -> one ksl
    stationary serves 4 heads; one vsl serves all q-blocks at a kb).
  - Attention runs fully "transposed": scoresT = ksl.T @ qT4 with keys on
    partitions; exp is merged into [128,1024] two-bank PSUM reads (one ACT
    instruction per 2 score tiles); only the diagonal tile gets a mask
    multiply. PV uses V in natural layout (via DMA-transpose from the
    projection) with a ones-column appended for the softmax denominator.
  - Softmax without max-subtraction: probs = exp(s/8); denominator from the
    ones-column; normalization applied to the PV output via
    reciprocal + partition_broadcast + elementwise multiply.
  - Matmuls in bf16; psums/softmax in fp32; partial output stored bf16.

kernel(**inputs) -> np.ndarray  takes full inputs, returns full [2,2048,2048].
"""

import functools
import os
import sys
import types

import numpy as np
import ml_dtypes

BF16 = ml_dtypes.bfloat16

B, S, D = 2, 2048, 2048
NH, NKV, HD = 32, 8, 64
BS = B * S               # 4096 rows total (b-major)
NB = S // 128            # 16 blocks per batch
NCORES = 8
VROW = HD + 1            # 65: [v | 1]


# --------------------------------------------------------------------------
# device graph (identical on all cores; per-core weights via input data)
# --------------------------------------------------------------------------

@functools.lru_cache(maxsize=None)
def _build_nc():
    import concourse.bacc as bacc
    import concourse.mybir as mybir
    import concourse.tile as tile

    BF = mybir.dt.bfloat16
    F32 = mybir.dt.float32
    EXP = mybir.ActivationFunctionType.Exp

    nc = bacc.Bacc(trn_type="TRN2", target_bir_lowering=False, debug=False,
                   num_devices=NCORES)

    dbg = bool(int(os.environ.get("KERNEL_DEBUG", "0")))
    if dbg:
        kT_dbg = nc.declare_dram_parameter("kT_dbg", [64, BS], BF,
                                           isOutput=True)
        vO_dbg = nc.declare_dram_parameter("vO_dbg", [128, 32 * 128], BF,
                                           isOutput=True)
        qT_dbg = nc.declare_dram_parameter("qT_dbg", [64, 32 * 512], BF,
                                           isOutput=True)
        aT_dbg = nc.declare_dram_parameter("aT_dbg", [2, 128, BS], BF,
                                           isOutput=True)

    xT_d = nc.declare_dram_parameter("xT", [D, BS], BF, isOutput=False)
    wq_d = nc.declare_dram_parameter("wq", [2, 128, 2048], BF, isOutput=False)
    wkv_d = nc.declare_dram_parameter("wkv", [128, 2048], BF, isOutput=False)
    wo_d = nc.declare_dram_parameter("wo", [2, 128, D], BF, isOutput=False)
    crep_d = nc.declare_dram_parameter("crep", [128, BS], BF, isOutput=False)
    ssig_d = nc.declare_dram_parameter("ssig", [128, BS], BF, isOutput=False)
    dmask_d = nc.declare_dram_parameter("dmask", [128, 512], BF, isOutput=False)
    out_d = nc.declare_dram_parameter("out", [BS, D], BF, isOutput=True)

    with tile.TileContext(nc) as tc:
        with tc.tile_pool(name="const", bufs=1) as cpool, \
             tc.tile_pool(name="persist", bufs=1) as ppool, \
             tc.tile_pool(name="xstream", bufs=32) as xpool, \
             tc.tile_pool(name="wstream", bufs=4) as wpool, \
             tc.tile_pool(name="work", bufs=3) as tpool, \
             tc.tile_pool(name="attn", bufs=3) as apool, \
             tc.tile_pool(name="ps", bufs=1, space="PSUM") as pspool:

            # ---- constants ----
            crep = cpool.tile([128, BS], BF, name="crep", tag="crep")
            nc.sync.dma_start(out=crep[:, :], in_=crep_d[:, :])
            ssig = cpool.tile([128, BS], BF, name="ssig", tag="ssig")
            nc.sync.dma_start(out=ssig[:, :], in_=ssig_d[:, :])
            dmask = cpool.tile([128, 512], BF, name="dmask", tag="dmask")
            nc.sync.dma_start(out=dmask[:, :], in_=dmask_d[:, :])
            wot = []
            for t in range(2):
                w_ = cpool.tile([128, D], BF, name=f"wot{t}", tag=f"wot{t}")
                nc.gpsimd.dma_start(out=w_[:, :], in_=wo_d[t, :, :])
                wot.append(w_)
            # resident projection weights (tiny: 2.5MB total)
            wkv_sb = cpool.tile([128, 2048], BF, name="wkv_sb", tag="wkv_sb")
            nc.sync.dma_start(out=wkv_sb[:, :], in_=wkv_d[:, :])
            wq_sb = []
            for hp in range(2):
                w_ = cpool.tile([128, 2048], BF, name=f"wq_sb{hp}",
                                tag=f"wq_sb{hp}")
                nc.sync.dma_start(out=w_[:, :], in_=wq_d[hp, :, :])
                wq_sb.append(w_)

            # ---- persistent activations ----
            kT = ppool.tile([64, BS], BF, name="kT", tag="kT")
            # V blocks padded to 128-col stride: dma_start_transpose needs
            # 128-byte-aligned output offsets (65-col stride corrupts).
            vOnes = ppool.tile([128, 32 * 128], BF, name="vOnes", tag="vOnes")
            vview = vOnes.rearrange("p (n w) -> p n w", w=128)
            nc.gpsimd.memset(vview[:, :, HD:HD + 1], 1.0)
            # qT4 cols: (b*16 + qb)*512 + h*128 + r   (h = head 0..3 local)
            qT4 = ppool.tile([64, 32 * 512], BF, name="qT4", tag="qT4")
            attnT = []
            for t in range(2):
                a_ = ppool.tile([128, BS], BF, name=f"attnT{t}", tag=f"attnT{t}")
                attnT.append(a_)

            # ---- projections (8 chunks of 512 rows; c = b*4 + g) ----
            kvAll = ppool.tile([128, BS], BF, name="kvAll", tag="kvAll")
            qAll = []
            for hp in range(2):
                q_ = ppool.tile([128, BS], BF, name=f"qAll{hp}",
                                tag=f"qAll{hp}")
                qAll.append(q_)
            for c in range(8):
                cs = c * 512
                xt = []
                for k in range(16):
                    t_ = xpool.tile([128, 512], BF, name="xt", tag="xt")
                    eng = nc.sync if k % 2 == 0 else nc.gpsimd
                    eng.dma_start(out=t_[:, :],
                                  in_=xT_d[k * 128:(k + 1) * 128, cs:cs + 512])
                    xt.append(t_)

                # K|V fused projection: psum rows 0:64 = kT, 64:128 = vT
                ps = pspool.tile([128, 512], F32, name="pskv", tag=f"pv{c % 4}")
                for k in range(16):
                    nc.tensor.matmul(ps[:, :],
                                     lhsT=wkv_sb[:, k * 128:(k + 1) * 128],
                                     rhs=xt[k][:, :],
                                     start=(k == 0), stop=(k == 15))
                nc.vector.tensor_copy(out=kvAll[:, cs:cs + 512], in_=ps[:, :])

                # Q projection
                for hp in range(2):
                    psq = pspool.tile([128, 512], F32, name="psq",
                                      tag=f"pv{(c + 2 + hp) % 4}")
                    for k in range(16):
                        nc.tensor.matmul(psq[:, :],
                                         lhsT=wq_sb[hp][:, k * 128:(k + 1) * 128],
                                         rhs=xt[k][:, :],
                                         start=(k == 0), stop=(k == 15))
                    if hp == 0:
                        nc.vector.tensor_copy(out=qAll[hp][:, cs:cs + 512],
                                              in_=psq[:, :])
                    else:
                        nc.scalar.copy(out=qAll[hp][:, cs:cs + 512],
                                       in_=psq[:, :])

            # ---- RoPE (wide, once) + V transposes ----
            # K: rows 0:64 of kvAll
            rotk = tpool.tile([64, BS], BF, name="rotk", tag="rotk", bufs=1)
            nc.gpsimd.dma_start(out=rotk[0:32, :], in_=kvAll[32:64, :])
            nc.sync.dma_start(out=rotk[32:64, :], in_=kvAll[0:32, :])
            nc.vector.tensor_mul(kT[0:64, :], kvAll[0:64, :], crep[0:64, :])
            nc.vector.tensor_mul(rotk[0:64, :], rotk[0:64, :], ssig[0:64, :])
            nc.vector.tensor_add(kT[0:64, :], kT[0:64, :], rotk[0:64, :])
            # V natural layout via DMA transpose (rows 64:128 of kvAll)
            for blk in range(32):
                nc.scalar.dma_start_transpose(
                    out=vOnes[:, blk * 128:blk * 128 + HD],
                    in_=kvAll[64:128, blk * 128:(blk + 1) * 128])
            # Q: per head-pair tile (in-place muls to save SBUF)
            qv = qT4.rearrange("p (blk h r) -> p blk h r", h=4, r=128)
            for hp in range(2):
                rotq = tpool.tile([128, BS], BF, name="rotq", tag="rotq",
                                  bufs=1)
                for i, (db, sb) in enumerate(
                        ((0, 32), (32, 0), (64, 96), (96, 64))):
                    eng = nc.sync if i % 2 == 0 else nc.gpsimd
                    eng.dma_start(out=rotq[db:db + 32, :],
                                  in_=qAll[hp][sb:sb + 32, :])
                nc.vector.tensor_mul(qAll[hp][:, :], qAll[hp][:, :],
                                     crep[:, :])
                nc.vector.tensor_mul(rotq[:, :], rotq[:, :], ssig[:, :])
                for ph in range(2):
                    h = 2 * hp + ph
                    dst = qv[:, :, h, :]
                    s2 = qAll[hp][ph * 64:(ph + 1) * 64, :].rearrange(
                        "p (j r) -> p j r", r=128)
                    s3 = rotq[ph * 64:(ph + 1) * 64, :].rearrange(
                        "p (j r) -> p j r", r=128)
                    nc.vector.tensor_add(dst, s2, s3)

            # ---- attention (causal, per (b, group-of-4-qbs)) ----
            def emit_pv(b, g, kb, scs, pvt):
                vbase = (b * 16 + kb) * 128
                for (pair, pr) in scs:
                    for jj, qb in enumerate(pair):
                        j = qb - 4 * g
                        nc.tensor.matmul(
                            pvt[j][0:VROW, :],
                            lhsT=vOnes[:, vbase:vbase + VROW],
                            rhs=pr[:, jj * 512:(jj + 1) * 512],
                            start=(kb == 0), stop=(kb == qb))
                        if kb == qb:
                            emit_norm(b, qb, pvt[j])

            def emit_norm(b, qb, pv):
                rsum = apool.tile([1, 512], F32, name="rsum", tag="rsum",
                                  bufs=1)
                nc.vector.tensor_copy(out=rsum[0:1, :], in_=pv[64:65, :])
                rsb = apool.tile([128, 512], F32, name="rsb", tag="rsb",
                                 bufs=1)
                nc.gpsimd.partition_broadcast(rsb[:, :], rsum[0:1, :])
                rep = apool.tile([128, 512], F32, name="rep", tag="rep",
                                 bufs=1)
                nc.vector.reciprocal_approx_fast(out=rep[:, :], in_=rsb[:, :])
                for t in range(2):
                    for ph in range(2):
                        h = 2 * t + ph
                        nc.vector.tensor_mul(
                            attnT[t][ph * 64:(ph + 1) * 64,
                                     b * S + qb * 128:b * S + (qb + 1) * 128],
                            pv[0:64, h * 128:(h + 1) * 128],
                            rep[ph * 64:(ph + 1) * 64, h * 128:(h + 1) * 128])

            for b in range(B):
                for g in range(4):
                    pvt = [pspool.tile([VROW, 512], F32, name=f"pv{j}",
                                       tag=f"pv{j}") for j in range(4)]
                    pend = []
                    for kb in range(4 * g + 4):
                        active = [qb for qb in range(4 * g, 4 * g + 4)
                                  if qb >= kb]
                        ksl = kT[0:64, b * S + kb * 128:b * S + (kb + 1) * 128]
                        scs = []
                        for ii in range(0, len(active), 2):
                            pair = active[ii:ii + 2]
                            sct = pspool.tile([128, 1024], F32, name="sc",
                                              tag="sc", bufs=2)
                            for jj, qb in enumerate(pair):
                                qs = (b * 16 + qb) * 512
                                nc.tensor.matmul(
                                    sct[:, jj * 512:(jj + 1) * 512],
                                    lhsT=ksl, rhs=qT4[0:64, qs:qs + 512],
                                    start=True, stop=True)
                            w = 512 * len(pair)
                            pr = apool.tile([128, 1024], BF, name="probs",
                                            tag="probs", bufs=8)
                            nc.scalar.activation(out=pr[:, 0:w],
                                                 in_=sct[:, 0:w],
                                                 func=EXP, scale=0.125)
                            scs.append((pair, pr))
                        if active and active[0] == kb:
                            pr0 = scs[0][1]
                            nc.vector.tensor_mul(pr0[:, 0:512], pr0[:, 0:512],
                                                 dmask[:, :])
                        pend.append((kb, scs))
                        if len(pend) > 2:
                            pk, ps_ = pend.pop(0)
                            emit_pv(b, g, pk, ps_, pvt)
                    for (pk, ps_) in pend:
                        emit_pv(b, g, pk, ps_, pvt)

            if dbg:
                nc.sync.dma_start(out=kT_dbg[:, :], in_=kT[0:64, :])
                nc.sync.dma_start(out=vO_dbg[:, :], in_=vOnes[:, :])
                nc.sync.dma_start(out=qT_dbg[:, :], in_=qT4[0:64, :])
                for t in range(2):
                    nc.sync.dma_start(out=aT_dbg[t, :, :], in_=attnT[t][:, :])

            # ---- output projection (partial: this core's 4 heads) ----
            for rb in range(32):
                po = [pspool.tile([128, 512], F32, name=f"po{dc}",
                                  tag=f"pv{dc}") for dc in range(4)]
                for t in range(2):
                    for dc in range(4):
                        nc.tensor.matmul(
                            po[dc][:, :],
                            lhsT=attnT[t][:, rb * 128:(rb + 1) * 128],
                            rhs=wot[t][:, dc * 512:(dc + 1) * 512],
                            start=(t == 0), stop=(t == 1))
                for dc in range(4):
                    ob = tpool.tile([128, 512], BF, name="ob", tag="ob",
                                    bufs=4)
                    if dc % 2 == 0:
                        nc.vector.tensor_copy(out=ob[:, :], in_=po[dc][:, :])
                    else:
                        nc.scalar.copy(out=ob[:, :], in_=po[dc][:, :])
                    eng = nc.sync if dc % 2 == 0 else nc.gpsimd
                    eng.dma_start(
                        out=out_d[rb * 128:(rb + 1) * 128,
                                  dc * 512:(dc + 1) * 512],
                        in_=ob[:, :])

    nc.compile()
    return nc


# --------------------------------------------------------------------------
# host-side sharding / layout prep
# --------------------------------------------------------------------------

def _deint(h):
    """de-interleaved column indices for head h (64 cols: evens then odds)."""
    return h * HD + np.concatenate([np.arange(0, HD, 2), np.arange(1, HD, 2)])


def _prep_shared(x, freqs_cos, freqs_sin, mask):
    xT = np.ascontiguousarray(
        x.reshape(BS, D).T).astype(BF16)                   # [D, BS]
    j = np.arange(128) % 32
    crep1 = freqs_cos[:, j].T                              # [128, S]
    crep = np.tile(crep1, (1, B)).astype(BF16)             # [128, BS]
    sgn = np.where((np.arange(128) // 32) % 2 == 0, -1.0, 1.0).astype(
        np.float32)
    ssig1 = freqs_sin[:, j].T * sgn[:, None]
    ssig = np.tile(ssig1, (1, B)).astype(BF16)
    dm = np.exp(mask[0:128, 0:128]).T                      # [128k, 128q]
    dmask = np.tile(dm, (1, 4)).astype(BF16)               # [128, 512]
    return xT, crep, ssig, dmask


def _prep_core(c, wq, wk, wv, wo):
    heads = [4 * c + h for h in range(4)]
    # wq: [2, 128, 2048] (head-pair, kdim-within-tile, k-tile*128 + head col)
    qcols = np.concatenate([_deint(h) for h in heads])     # [256]
    wq_c = wq[:, qcols].reshape(16, 128, 2, 128).transpose(2, 1, 0, 3)
    wq_c = np.ascontiguousarray(wq_c.reshape(2, 128, 2048)).astype(BF16)
    # wkv: [128, 2048] = per k-tile 128 cols: [wk deint 64 | wv natural 64]
    kcols = _deint(c)
    vcols = c * HD + np.arange(HD)
    wkv = np.concatenate([wk[:, kcols], wv[:, vcols]], axis=1)  # [D, 128]
    wkv_c = wkv.reshape(16, 128, 128).transpose(1, 0, 2)
    wkv_c = np.ascontiguousarray(wkv_c.reshape(128, 2048)).astype(BF16)
    # wo: [2, 128, D] rows = heads 2t, 2t+1 natural hd
    worows = np.concatenate([h * HD + np.arange(HD) for h in heads])
    wo_c = np.ascontiguousarray(wo[worows, :].reshape(2, 128, D)).astype(BF16)
    return wq_c, wkv_c, wo_c


def _assemble(results):
    acc = np.zeros((BS, D), np.float32)
    for i in range(NCORES):
        acc += np.asarray(results[i]["out"], dtype=np.float32)
    return np.ascontiguousarray(acc.reshape(B, S, D))


LAST_RUN_INFO = {}


def kernel(x, freqs_cos, freqs_sin, mask, wq, wk, wv, wo, start_pos=0):
    from concourse.bass_utils import run_bass_kernel_spmd

    x = np.asarray(x, dtype=np.float32)
    freqs_cos = np.asarray(freqs_cos, dtype=np.float32)
    freqs_sin = np.asarray(freqs_sin, dtype=np.float32)
    mask = np.asarray(mask, dtype=np.float32)
    wq = np.asarray(wq, dtype=np.float32)
    wk = np.asarray(wk, dtype=np.float32)
    wv = np.asarray(wv, dtype=np.float32)
    wo = np.asarray(wo, dtype=np.float32)

    xT, crep, ssig, dmask = _prep_shared(x, freqs_cos, freqs_sin, mask)
    in_maps = []
    for c in range(NCORES):
        wq_c, wkv_c, wo_c = _prep_core(c, wq, wk, wv, wo)
        in_maps.append({
            "xT": xT, "wq": wq_c, "wkv": wkv_c, "wo": wo_c,
            "crep": crep, "ssig": ssig, "dmask": dmask,
        })

    nc = _build_nc()

    trace = bool(int(os.environ.get("KERNEL_TRACE", "0")))
    kwargs = {}
    if trace:
        _install_ntff_hook()
        import concourse.bass_utils as bass_utils
        bass_utils.upload_artifacts = lambda tmpdir: tmpdir
        import tempfile
        tmpdir = tempfile.mkdtemp(prefix="attn_trace_")
        kwargs = {"trace": True, "tmpdir": tmpdir}

    res = run_bass_kernel_spmd(nc, in_maps, core_ids=list(range(NCORES)),
                               **kwargs)
    LAST_RUN_INFO.clear()
    LAST_RUN_INFO.update({
        "exec_time_ns": res.exec_time_ns,
        "tmpdir": kwargs.get("tmpdir"),
        "res": res,
    })
    return _assemble(res.results)


def _install_ntff_hook():
    if "antenv.axon_hooks" not in sys.modules:
        import antenv

        mod = types.ModuleType("antenv.axon_hooks")
        mod._hook = None
        mod.set_axon_ntff_profile_hook = lambda h: setattr(mod, "_hook", h)
        mod.get_axon_ntff_profile_hook = lambda: mod._hook
        sys.modules["antenv.axon_hooks"] = mod
        antenv.axon_hooks = mod
    from trn_agent_boot.trn_boot import _ntff_profile_via_ctypes
    from antenv.axon_hooks import set_axon_ntff_profile_hook as _set

    _set(_ntff_profile_via_ctypes("/opt/axon/libaxon_pjrt.so"))


# revision 15
# speedup vs baseline: 1.2580x; 1.2580x over previous
"""Distributed GQA attention kernel for 8 TRN2 NeuronCores.

Problem: B=2, S=2048, D=2048, 32 q-heads / 8 kv-heads, hd=64, causal + RoPE.

Strategy (kv-head tensor parallel, zero collectives):
  - Core c owns kv-head c (q-heads 4c..4c+3) for BOTH batches over ALL rows.
    Every core loads the full x (host-pretransposed to xT bf16) and projects
    Q (4 heads), K, V (1 kv head each) for all 4096 rows. K/V never leave the
    core, so there are NO collectives. Each core computes a PARTIAL output
    (its 4 heads x its 256 wo rows) and the host sums the 8 partials.
  - Because every core sees all rows, the causal structure is IDENTICAL on
    all cores (SPMD-compatible): per q-block qb only key blocks kb <= qb are
    computed -> ~47% of score/exp/PV work skipped exactly, with all matmuls
    staying N=512 wide (4 q-heads x 128 rows share one kv head -> one ksl
    stationary serves 4 heads; one vsl serves all q-blocks at a kb).
  - Attention runs fully "transposed": scoresT = ksl.T @ qT4 with keys on
    partitions; exp is merged into [128,1024] two-bank PSUM reads (one ACT
    instruction per 2 score tiles); only the diagonal tile gets a mask
    multiply. PV uses V in natural layout (via DMA-transpose from the
    projection) with a ones-column appended for the softmax denominator.
  - Softmax without max-subtraction: probs = exp(s/8); denominator from the
    ones-column; normalization applied to the PV output via
    reciprocal + partition_broadcast + elementwise multiply.
  - Matmuls in bf16; psums/softmax in fp32; partial output stored bf16.

kernel(**inputs) -> np.ndarray  takes full inputs, returns full [2,2048,2048].
"""

import functools
import os
import sys
import types

import numpy as np
import ml_dtypes

BF16 = ml_dtypes.bfloat16

B, S, D = 2, 2048, 2048
NH, NKV, HD = 32, 8, 64
BS = B * S               # 4096 rows total (b-major)
NB = S // 128            # 16 blocks per batch
NCORES = 8
VROW = HD + 1            # 65: [v | 1]


# --------------------------------------------------------------------------
# device graph (identical on all cores; per-core weights via input data)
# --------------------------------------------------------------------------

@functools.lru_cache(maxsize=None)
def _build_nc():
    import concourse.bacc as bacc
    import concourse.mybir as mybir
    import concourse.tile as tile

    BF = mybir.dt.bfloat16
    F32 = mybir.dt.float32
    EXP = mybir.ActivationFunctionType.Exp

    nc = bacc.Bacc(trn_type="TRN2", target_bir_lowering=False, debug=False,
                   num_devices=NCORES)

    dbg = bool(int(os.environ.get("KERNEL_DEBUG", "0")))
    if dbg:
        kT_dbg = nc.declare_dram_parameter("kT_dbg", [64, BS], BF,
                                           isOutput=True)
        vO_dbg = nc.declare_dram_parameter("vO_dbg", [128, 32 * 128], BF,
                                           isOutput=True)
        qT_dbg = nc.declare_dram_parameter("qT_dbg", [64, 32 * 512], BF,
                                           isOutput=True)
        aT_dbg = nc.declare_dram_parameter("aT_dbg", [2, 128, BS], BF,
                                           isOutput=True)

    xT_d = nc.declare_dram_parameter("xT", [D, BS], BF, isOutput=False)
    wq_d = nc.declare_dram_parameter("wq", [2, 128, 2048], BF, isOutput=False)
    wkv_d = nc.declare_dram_parameter("wkv", [128, 2048], BF, isOutput=False)
    wo_d = nc.declare_dram_parameter("wo", [2, 128, D], BF, isOutput=False)
    crep_d = nc.declare_dram_parameter("crep", [128, BS], BF, isOutput=False)
    ssig_d = nc.declare_dram_parameter("ssig", [128, BS], BF, isOutput=False)
    dmask_d = nc.declare_dram_parameter("dmask", [128, 512], BF, isOutput=False)
    out_d = nc.declare_dram_parameter("out", [BS, D], BF, isOutput=True)

    with tile.TileContext(nc) as tc:
        with tc.tile_pool(name="const", bufs=1) as cpool, \
             tc.tile_pool(name="persist", bufs=1) as ppool, \
             tc.tile_pool(name="xstream", bufs=27) as xpool, \
             tc.tile_pool(name="wstream", bufs=4) as wpool, \
             tc.tile_pool(name="work", bufs=3) as tpool, \
             tc.tile_pool(name="attn", bufs=3) as apool, \
             tc.tile_pool(name="ps", bufs=1, space="PSUM") as pspool:

            # ---- constants ----
            crep = cpool.tile([128, BS], BF, name="crep", tag="crep")
            nc.sync.dma_start(out=crep[:, :], in_=crep_d[:, :])
            ssig = cpool.tile([128, BS], BF, name="ssig", tag="ssig")
            nc.sync.dma_start(out=ssig[:, :], in_=ssig_d[:, :])
            dmask = cpool.tile([128, 512], BF, name="dmask", tag="dmask")
            nc.sync.dma_start(out=dmask[:, :], in_=dmask_d[:, :])
            wot = []
            for t in range(2):
                w_ = cpool.tile([128, D], BF, name=f"wot{t}", tag=f"wot{t}")
                nc.gpsimd.dma_start(out=w_[:, :], in_=wo_d[t, :, :])
                wot.append(w_)
            # resident projection weights (tiny: 2.5MB total)
            wkv_sb = cpool.tile([128, 2048], BF, name="wkv_sb", tag="wkv_sb")
            nc.sync.dma_start(out=wkv_sb[:, :], in_=wkv_d[:, :])
            wq_sb = []
            for hp in range(2):
                w_ = cpool.tile([128, 2048], BF, name=f"wq_sb{hp}",
                                tag=f"wq_sb{hp}")
                nc.sync.dma_start(out=w_[:, :], in_=wq_d[hp, :, :])
                wq_sb.append(w_)

            # ---- persistent activations ----
            kT = ppool.tile([64, BS], BF, name="kT", tag="kT")
            # V blocks padded to 128-col stride: dma_start_transpose needs
            # 128-byte-aligned output offsets (65-col stride corrupts).
            vOnes = ppool.tile([128, 32 * 128], BF, name="vOnes", tag="vOnes")
            vview = vOnes.rearrange("p (n w) -> p n w", w=128)
            nc.gpsimd.memset(vview[:, :, HD:HD + 1], 1.0)
            # qT4 cols: (b*16 + qb)*512 + h*128 + r   (h = head 0..3 local)
            qT4 = ppool.tile([64, 32 * 512], BF, name="qT4", tag="qT4")
            attnT = []
            for t in range(2):
                a_ = ppool.tile([128, BS], BF, name=f"attnT{t}", tag=f"attnT{t}")
                attnT.append(a_)

            # ---- projections (8 chunks of 512 rows; c = b*4 + g) ----
            qv = qT4.rearrange("p (blk h r) -> p blk h r", h=4, r=128)
            for c in range(8):
                b, g = divmod(c, 4)
                cs = c * 512
                xt = []
                for k in range(16):
                    t_ = xpool.tile([128, 512], BF, name="xt", tag="xt")
                    eng = nc.sync if k % 2 == 0 else nc.gpsimd
                    eng.dma_start(out=t_[:, :],
                                  in_=xT_d[k * 128:(k + 1) * 128, cs:cs + 512])
                    xt.append(t_)

                # K|V fused projection: psum rows 0:64 = kT, 64:128 = vT
                ps = pspool.tile([128, 512], F32, name="pskv", tag=f"pv{c % 4}")
                for k in range(16):
                    nc.tensor.matmul(ps[:, :],
                                     lhsT=wkv_sb[:, k * 128:(k + 1) * 128],
                                     rhs=xt[k][:, :],
                                     start=(k == 0), stop=(k == 15))
                kvraw = tpool.tile([128, 512], BF, name="kvraw", tag="kvraw", bufs=6)
                nc.vector.tensor_copy(out=kvraw[:, :], in_=ps[:, :])
                # K rope (rows 0:64; [ev32|od32] de-interleaved)
                rot = tpool.tile([64, 512], BF, name="rot", tag="rot")
                nc.scalar.dma_start(out=rot[0:32, :], in_=kvraw[32:64, :])
                nc.scalar.dma_start(out=rot[32:64, :], in_=kvraw[0:32, :])
                nc.vector.tensor_mul(kT[0:64, cs:cs + 512], kvraw[0:64, :],
                                     crep[0:64, cs:cs + 512])
                nc.vector.tensor_mul(rot[0:64, :], rot[0:64, :],
                                     ssig[0:64, cs:cs + 512])
                nc.vector.tensor_add(kT[0:64, cs:cs + 512],
                                     kT[0:64, cs:cs + 512], rot[0:64, :])
                # V natural layout via DMA transpose (rows 64:128 of kvraw)
                for j in range(4):
                    blk = c * 4 + j
                    nc.scalar.dma_start_transpose(
                        out=vOnes[:, blk * 128:blk * 128 + HD],
                        in_=kvraw[64:128, j * 128:(j + 1) * 128])

                # Q projection + rope -> qT4
                for hp in range(2):
                    psq = pspool.tile([128, 512], F32, name="psq",
                                      tag=f"pv{(c + 2 + hp) % 4}")
                    for k in range(16):
                        nc.tensor.matmul(
                            psq[:, :],
                            lhsT=wq_sb[hp][:, k * 128:(k + 1) * 128],
                            rhs=xt[k][:, :],
                            start=(k == 0), stop=(k == 15))
                    qraw = tpool.tile([128, 512], BF, name="qraw", tag="qraw")
                    nc.vector.tensor_copy(out=qraw[:, :], in_=psq[:, :])
                    rotq = tpool.tile([128, 512], BF, name="rotq", tag="rotq")
                    for i, (db, sb) in enumerate(
                            ((0, 32), (32, 0), (64, 96), (96, 64))):
                        eng = nc.scalar if (i + hp) % 2 == 0 else nc.gpsimd
                        eng.dma_start(out=rotq[db:db + 32, :],
                                      in_=qraw[sb:sb + 32, :])
                    q2 = tpool.tile([128, 512], BF, name="q2", tag="q2")
                    nc.vector.tensor_mul(q2[:, :], qraw[:, :],
                                         crep[:, cs:cs + 512])
                    nc.vector.tensor_mul(rotq[:, :], rotq[:, :],
                                         ssig[:, cs:cs + 512])
                    for ph in range(2):
                        h = 2 * hp + ph
                        dst = qv[:, b * 16 + 4 * g:b * 16 + 4 * g + 4, h, :]
                        s2 = q2[ph * 64:(ph + 1) * 64, :].rearrange(
                            "p (j r) -> p j r", r=128)
                        s3 = rotq[ph * 64:(ph + 1) * 64, :].rearrange(
                            "p (j r) -> p j r", r=128)
                        nc.vector.tensor_add(dst, s2, s3)

            # ---- attention (causal, per (b, group-of-4-qbs)) ----
            def emit_pv(b, g, kb, scs, pvt):
                vbase = (b * 16 + kb) * 128
                for (pair, pr) in scs:
                    for jj, qb in enumerate(pair):
                        j = qb - 4 * g
                        nc.tensor.matmul(
                            pvt[j][0:VROW, :],
                            lhsT=vOnes[:, vbase:vbase + VROW],
                            rhs=pr[:, jj * 512:(jj + 1) * 512],
                            start=(kb == 0), stop=(kb == qb))
                        if kb == qb:
                            emit_norm(b, qb, pvt[j])

            def emit_norm(b, qb, pv):
                rsum = apool.tile([1, 512], F32, name="rsum", tag="rsum",
                                  bufs=2)
                nc.vector.tensor_copy(out=rsum[0:1, :], in_=pv[64:65, :])
                rsb = apool.tile([128, 512], F32, name="rsb", tag="rsb",
                                 bufs=2)
                nc.gpsimd.partition_broadcast(rsb[:, :], rsum[0:1, :])
                rep = apool.tile([128, 512], F32, name="rep", tag="rep",
                                 bufs=2)
                nc.vector.reciprocal_approx_fast(out=rep[:, :], in_=rsb[:, :])
                for t in range(2):
                    for ph in range(2):
                        h = 2 * t + ph
                        nc.vector.tensor_mul(
                            attnT[t][ph * 64:(ph + 1) * 64,
                                     b * S + qb * 128:b * S + (qb + 1) * 128],
                            pv[0:64, h * 128:(h + 1) * 128],
                            rep[ph * 64:(ph + 1) * 64, h * 128:(h + 1) * 128])

            for b in range(B):
                for g in range(4):
                    pvt = [pspool.tile([VROW, 512], F32, name=f"pv{j}",
                                       tag=f"pv{j}") for j in range(4)]
                    pend = []
                    for kb in range(4 * g + 4):
                        active = [qb for qb in range(4 * g, 4 * g + 4)
                                  if qb >= kb]
                        ksl = kT[0:64, b * S + kb * 128:b * S + (kb + 1) * 128]
                        scs = []
                        for ii in range(0, len(active), 2):
                            pair = active[ii:ii + 2]
                            sct = pspool.tile([128, 1024], F32, name="sc",
                                              tag="sc", bufs=2)
                            for jj, qb in enumerate(pair):
                                qs = (b * 16 + qb) * 512
                                nc.tensor.matmul(
                                    sct[:, jj * 512:(jj + 1) * 512],
                                    lhsT=ksl, rhs=qT4[0:64, qs:qs + 512],
                                    start=True, stop=True)
                            w = 512 * len(pair)
                            pr = apool.tile([128, 1024], BF, name="probs",
                                            tag="probs", bufs=6)
                            nc.scalar.activation(out=pr[:, 0:w],
                                                 in_=sct[:, 0:w],
                                                 func=EXP, scale=0.125)
                            scs.append((pair, pr))
                        if active and active[0] == kb:
                            pr0 = scs[0][1]
                            nc.vector.tensor_mul(pr0[:, 0:512], pr0[:, 0:512],
                                                 dmask[:, :])
                        pend.append((kb, scs))
                        if len(pend) > 2:
                            pk, ps_ = pend.pop(0)
                            emit_pv(b, g, pk, ps_, pvt)
                    for (pk, ps_) in pend:
                        emit_pv(b, g, pk, ps_, pvt)

            if dbg:
                nc.sync.dma_start(out=kT_dbg[:, :], in_=kT[0:64, :])
                nc.sync.dma_start(out=vO_dbg[:, :], in_=vOnes[:, :])
                nc.sync.dma_start(out=qT_dbg[:, :], in_=qT4[0:64, :])
                for t in range(2):
                    nc.sync.dma_start(out=aT_dbg[t, :, :], in_=attnT[t][:, :])

            # ---- output projection (partial: this core's 4 heads) ----
            # alternate wide (sc-tag, 2 banks) and narrow (pv-tag) psum groups
            # so 6 accumulation groups stay in flight.
            for rb in range(32):
                if rb % 2 == 0:
                    po = [pspool.tile([128, 1024], F32, name=f"po{dh}",
                                      tag="sc", bufs=2) for dh in range(2)]
                    tgt = [(po[0][:, 0:512], 0), (po[0][:, 512:1024], 1),
                           (po[1][:, 0:512], 2), (po[1][:, 512:1024], 3)]
                else:
                    po = [pspool.tile([128, 512], F32, name=f"po{dc}",
                                      tag=f"pv{dc}") for dc in range(4)]
                    tgt = [(po[dc][:, :], dc) for dc in range(4)]
                for t in range(2):
                    for (dst, dc) in tgt:
                        nc.tensor.matmul(
                            dst,
                            lhsT=attnT[t][:, rb * 128:(rb + 1) * 128],
                            rhs=wot[t][:, dc * 512:(dc + 1) * 512],
                            start=(t == 0), stop=(t == 1))
                if rb % 2 == 0:
                    for dh in range(2):
                        ob = tpool.tile([128, 1024], BF, name="ob", tag="ob",
                                        bufs=4)
                        if dh % 2 == 0:
                            nc.vector.tensor_copy(out=ob[:, :],
                                                  in_=po[dh][:, :])
                        else:
                            nc.scalar.copy(out=ob[:, :], in_=po[dh][:, :])
                        eng = nc.sync if dh % 2 == 0 else nc.gpsimd
                        eng.dma_start(
                            out=out_d[rb * 128:(rb + 1) * 128,
                                      dh * 1024:(dh + 1) * 1024],
                            in_=ob[:, :])
                else:
                    for dc in range(4):
                        ob = tpool.tile([128, 512], BF, name="ob2", tag="ob2",
                                        bufs=4)
                        if dc % 2 == 0:
                            nc.vector.tensor_copy(out=ob[:, :],
                                                  in_=po[dc][:, :])
                        else:
                            nc.scalar.copy(out=ob[:, :], in_=po[dc][:, :])
                        eng = nc.sync if dc % 2 == 0 else nc.gpsimd
                        eng.dma_start(
                            out=out_d[rb * 128:(rb + 1) * 128,
                                      dc * 512:(dc + 1) * 512],
                            in_=ob[:, :])

    nc.compile()
    return nc


# --------------------------------------------------------------------------
# host-side sharding / layout prep
# --------------------------------------------------------------------------

def _deint(h):
    """de-interleaved column indices for head h (64 cols: evens then odds)."""
    return h * HD + np.concatenate([np.arange(0, HD, 2), np.arange(1, HD, 2)])


def _prep_shared(x, freqs_cos, freqs_sin, mask):
    xT = np.ascontiguousarray(
        x.reshape(BS, D).T).astype(BF16)                   # [D, BS]
    j = np.arange(128) % 32
    crep1 = freqs_cos[:, j].T                              # [128, S]
    crep = np.tile(crep1, (1, B)).astype(BF16)             # [128, BS]
    sgn = np.where((np.arange(128) // 32) % 2 == 0, -1.0, 1.0).astype(
        np.float32)
    ssig1 = freqs_sin[:, j].T * sgn[:, None]
    ssig = np.tile(ssig1, (1, B)).astype(BF16)
    dm = np.exp(mask[0:128, 0:128]).T                      # [128k, 128q]
    dmask = np.tile(dm, (1, 4)).astype(BF16)               # [128, 512]
    return xT, crep, ssig, dmask


def _prep_core(c, wq, wk, wv, wo):
    heads = [4 * c + h for h in range(4)]
    # wq: [2, 128, 2048] (head-pair, kdim-within-tile, k-tile*128 + head col)
    qcols = np.concatenate([_deint(h) for h in heads])     # [256]
    wq_c = wq[:, qcols].reshape(16, 128, 2, 128).transpose(2, 1, 0, 3)
    wq_c = np.ascontiguousarray(wq_c.reshape(2, 128, 2048)).astype(BF16)
    # wkv: [128, 2048] = per k-tile 128 cols: [wk deint 64 | wv natural 64]
    kcols = _deint(c)
    vcols = c * HD + np.arange(HD)
    wkv = np.concatenate([wk[:, kcols], wv[:, vcols]], axis=1)  # [D, 128]
    wkv_c = wkv.reshape(16, 128, 128).transpose(1, 0, 2)
    wkv_c = np.ascontiguousarray(wkv_c.reshape(128, 2048)).astype(BF16)
    # wo: [2, 128, D] rows = heads 2t, 2t+1 natural hd
    worows = np.concatenate([h * HD + np.arange(HD) for h in heads])
    wo_c = np.ascontiguousarray(wo[worows, :].reshape(2, 128, D)).astype(BF16)
    return wq_c, wkv_c, wo_c


def _assemble(results):
    acc = np.zeros((BS, D), np.float32)
    for i in range(NCORES):
        acc += np.asarray(results[i]["out"], dtype=np.float32)
    return np.ascontiguousarray(acc.reshape(B, S, D))


LAST_RUN_INFO = {}


def kernel(x, freqs_cos, freqs_sin, mask, wq, wk, wv, wo, start_pos=0):
    from concourse.bass_utils import run_bass_kernel_spmd

    x = np.asarray(x, dtype=np.float32)
    freqs_cos = np.asarray(freqs_cos, dtype=np.float32)
    freqs_sin = np.asarray(freqs_sin, dtype=np.float32)
    mask = np.asarray(mask, dtype=np.float32)
    wq = np.asarray(wq, dtype=np.float32)
    wk = np.asarray(wk, dtype=np.float32)
    wv = np.asarray(wv, dtype=np.float32)
    wo = np.asarray(wo, dtype=np.float32)

    xT, crep, ssig, dmask = _prep_shared(x, freqs_cos, freqs_sin, mask)
    in_maps = []
    for c in range(NCORES):
        wq_c, wkv_c, wo_c = _prep_core(c, wq, wk, wv, wo)
        in_maps.append({
            "xT": xT, "wq": wq_c, "wkv": wkv_c, "wo": wo_c,
            "crep": crep, "ssig": ssig, "dmask": dmask,
        })

    nc = _build_nc()

    trace = bool(int(os.environ.get("KERNEL_TRACE", "0")))
    kwargs = {}
    if trace:
        _install_ntff_hook()
        import concourse.bass_utils as bass_utils
        bass_utils.upload_artifacts = lambda tmpdir: tmpdir
        import tempfile
        tmpdir = tempfile.mkdtemp(prefix="attn_trace_")
        kwargs = {"trace": True, "tmpdir": tmpdir}

    res = run_bass_kernel_spmd(nc, in_maps, core_ids=list(range(NCORES)),
                               **kwargs)
    LAST_RUN_INFO.clear()
    LAST_RUN_INFO.update({
        "exec_time_ns": res.exec_time_ns,
        "tmpdir": kwargs.get("tmpdir"),
        "res": res,
    })
    return _assemble(res.results)


def _install_ntff_hook():
    if "antenv.axon_hooks" not in sys.modules:
        import antenv

        mod = types.ModuleType("antenv.axon_hooks")
        mod._hook = None
        mod.set_axon_ntff_profile_hook = lambda h: setattr(mod, "_hook", h)
        mod.get_axon_ntff_profile_hook = lambda: mod._hook
        sys.modules["antenv.axon_hooks"] = mod
        antenv.axon_hooks = mod
    from trn_agent_boot.trn_boot import _ntff_profile_via_ctypes
    from antenv.axon_hooks import set_axon_ntff_profile_hook as _set

    _set(_ntff_profile_via_ctypes("/opt/axon/libaxon_pjrt.so"))


# revision 17
# speedup vs baseline: 1.3058x; 1.0380x over previous
"""Distributed GQA attention kernel for 8 TRN2 NeuronCores.

Problem: B=2, S=2048, D=2048, 32 q-heads / 8 kv-heads, hd=64, causal + RoPE.

Strategy (kv-head tensor parallel, zero collectives):
  - Core c owns kv-head c (q-heads 4c..4c+3) for BOTH batches over ALL rows.
    Every core loads the full x (host-pretransposed to xT bf16) and projects
    Q (4 heads), K, V (1 kv head each) for all 4096 rows. K/V never leave the
    core, so there are NO collectives. Each core computes a PARTIAL output
    (its 4 heads x its 256 wo rows) and the host sums the 8 partials.
  - Because every core sees all rows, the causal structure is IDENTICAL on
    all cores (SPMD-compatible): per q-block qb only key blocks kb <= qb are
    computed -> ~47% of score/exp/PV work skipped exactly, with all matmuls
    staying N=512 wide (4 q-heads x 128 rows share one kv head -> one ksl
    stationary serves 4 heads; one vsl serves all q-blocks at a kb).
  - Attention runs fully "transposed": scoresT = ksl.T @ qT4 with keys on
    partitions; exp is merged into [128,1024] two-bank PSUM reads (one ACT
    instruction per 2 score tiles); only the diagonal tile gets a mask
    multiply. PV uses V in natural layout (via DMA-transpose from the
    projection) with a ones-column appended for the softmax denominator.
  - Softmax without max-subtraction: probs = exp(s/8); denominator from the
    ones-column; normalization applied to the PV output via
    reciprocal + partition_broadcast + elementwise multiply.
  - Matmuls in bf16; psums/softmax in fp32; partial output stored bf16.

kernel(**inputs) -> np.ndarray  takes full inputs, returns full [2,2048,2048].
"""

import functools
import os
import sys
import types

import numpy as np
import ml_dtypes

BF16 = ml_dtypes.bfloat16

B, S, D = 2, 2048, 2048
NH, NKV, HD = 32, 8, 64
BS = B * S               # 4096 rows total (b-major)
NB = S // 128            # 16 blocks per batch
NCORES = 8
VROW = HD + 1            # 65: [v | 1]


# --------------------------------------------------------------------------
# device graph (identical on all cores; per-core weights via input data)
# --------------------------------------------------------------------------

@functools.lru_cache(maxsize=None)
def _build_nc():
    import concourse.bacc as bacc
    import concourse.mybir as mybir
    import concourse.tile as tile

    BF = mybir.dt.bfloat16
    F32 = mybir.dt.float32
    EXP = mybir.ActivationFunctionType.Exp

    nc = bacc.Bacc(trn_type="TRN2", target_bir_lowering=False, debug=False,
                   num_devices=NCORES)

    dbg = bool(int(os.environ.get("KERNEL_DEBUG", "0")))
    if dbg:
        kT_dbg = nc.declare_dram_parameter("kT_dbg", [64, BS], BF,
                                           isOutput=True)
        vO_dbg = nc.declare_dram_parameter("vO_dbg", [128, 32 * 128], BF,
                                           isOutput=True)
        qT_dbg = nc.declare_dram_parameter("qT_dbg", [64, 32 * 512], BF,
                                           isOutput=True)
        aT_dbg = nc.declare_dram_parameter("aT_dbg", [2, 128, BS], BF,
                                           isOutput=True)

    xT_d = nc.declare_dram_parameter("xT", [D, BS], BF, isOutput=False)
    wq_d = nc.declare_dram_parameter("wq", [2, 128, 2048], BF, isOutput=False)
    wkv_d = nc.declare_dram_parameter("wkv", [128, 2048], BF, isOutput=False)
    wo_d = nc.declare_dram_parameter("wo", [2, 128, D], BF, isOutput=False)
    crep_d = nc.declare_dram_parameter("crep", [128, BS], BF, isOutput=False)
    ssig_d = nc.declare_dram_parameter("ssig", [128, BS], BF, isOutput=False)
    dmask_d = nc.declare_dram_parameter("dmask", [128, 512], BF, isOutput=False)
    out_d = nc.declare_dram_parameter("out", [BS, D], BF, isOutput=True)

    with tile.TileContext(nc) as tc:
        with tc.tile_pool(name="const", bufs=1) as cpool, \
             tc.tile_pool(name="persist", bufs=1) as ppool, \
             tc.tile_pool(name="xstream", bufs=27) as xpool, \
             tc.tile_pool(name="wstream", bufs=4) as wpool, \
             tc.tile_pool(name="work", bufs=3) as tpool, \
             tc.tile_pool(name="attn", bufs=3) as apool, \
             tc.tile_pool(name="ps", bufs=1, space="PSUM") as pspool:

            # ---- constants ----
            crep = cpool.tile([128, BS], BF, name="crep", tag="crep")
            nc.sync.dma_start(out=crep[:, :], in_=crep_d[:, :])
            ssig = cpool.tile([128, BS], BF, name="ssig", tag="ssig")
            nc.sync.dma_start(out=ssig[:, :], in_=ssig_d[:, :])
            dmask = cpool.tile([128, 512], BF, name="dmask", tag="dmask")
            nc.sync.dma_start(out=dmask[:, :], in_=dmask_d[:, :])
            wot = []
            for t in range(2):
                w_ = cpool.tile([128, D], BF, name=f"wot{t}", tag=f"wot{t}")
                nc.gpsimd.dma_start(out=w_[:, :], in_=wo_d[t, :, :])
                wot.append(w_)
            # resident projection weights (tiny: 2.5MB total)
            wkv_sb = cpool.tile([128, 2048], BF, name="wkv_sb", tag="wkv_sb")
            nc.sync.dma_start(out=wkv_sb[:, :], in_=wkv_d[:, :])
            wq_sb = []
            for hp in range(2):
                w_ = cpool.tile([128, 2048], BF, name=f"wq_sb{hp}",
                                tag=f"wq_sb{hp}")
                nc.sync.dma_start(out=w_[:, :], in_=wq_d[hp, :, :])
                wq_sb.append(w_)

            # ---- persistent activations ----
            kT = ppool.tile([64, BS], BF, name="kT", tag="kT")
            # V blocks padded to 128-col stride: dma_start_transpose needs
            # 128-byte-aligned output offsets (65-col stride corrupts).
            vOnes = ppool.tile([128, 32 * 128], BF, name="vOnes", tag="vOnes")
            vview = vOnes.rearrange("p (n w) -> p n w", w=128)
            nc.gpsimd.memset(vview[:, :, HD:HD + 1], 1.0)
            # qT4 cols: (b*16 + qb)*512 + h*128 + r   (h = head 0..3 local)
            qT4 = ppool.tile([64, 32 * 512], BF, name="qT4", tag="qT4")
            attnT = []
            for t in range(2):
                a_ = ppool.tile([128, BS], BF, name=f"attnT{t}", tag=f"attnT{t}")
                attnT.append(a_)

            # ---- interleaved projections + attention ----
            # Chunk c projects rows [c*512, (c+1)*512) (c = b*4 + g).
            # Attention runs in groups of 2 q-blocks (pv0/pv1 psum tags);
            # group (b, m) covers qb in {2m, 2m+1} and only needs chunks
            # <= b*4 + (2m+1)//4, so groups interleave with later chunks,
            # keeping PE dense through the whole kernel.
            qv = qT4.rearrange("p (blk h r) -> p blk h r", h=4, r=128)

            def proj_chunk(c):
                cs = c * 512
                b, g = divmod(c, 4)
                xt = []
                for k in range(16):
                    t_ = xpool.tile([128, 512], BF, name="xt", tag="xt")
                    eng = nc.sync if k % 2 == 0 else nc.gpsimd
                    eng.dma_start(out=t_[:, :],
                                  in_=xT_d[k * 128:(k + 1) * 128, cs:cs + 512])
                    xt.append(t_)

                # K|V fused projection: psum rows 0:64 = kT, 64:128 = vT
                ps = pspool.tile([128, 512], F32, name="pskv", tag="pj",
                                 bufs=2)
                for k in range(16):
                    nc.tensor.matmul(ps[:, :],
                                     lhsT=wkv_sb[:, k * 128:(k + 1) * 128],
                                     rhs=xt[k][:, :],
                                     start=(k == 0), stop=(k == 15))
                kvraw = tpool.tile([128, 512], BF, name="kvraw", tag="kvraw",
                                   bufs=6)
                nc.vector.tensor_copy(out=kvraw[:, :], in_=ps[:, :])
                # K rope (rows 0:64; [ev32|od32] de-interleaved)
                rot = tpool.tile([64, 512], BF, name="rot", tag="rot")
                nc.scalar.dma_start(out=rot[0:32, :], in_=kvraw[32:64, :])
                nc.scalar.dma_start(out=rot[32:64, :], in_=kvraw[0:32, :])
                nc.vector.tensor_mul(kT[0:64, cs:cs + 512], kvraw[0:64, :],
                                     crep[0:64, cs:cs + 512])
                nc.vector.tensor_mul(rot[0:64, :], rot[0:64, :],
                                     ssig[0:64, cs:cs + 512])
                nc.vector.tensor_add(kT[0:64, cs:cs + 512],
                                     kT[0:64, cs:cs + 512], rot[0:64, :])
                # V natural layout via DMA transpose (rows 64:128 of kvraw)
                for j in range(4):
                    blk = c * 4 + j
                    nc.scalar.dma_start_transpose(
                        out=vOnes[:, blk * 128:blk * 128 + HD],
                        in_=kvraw[64:128, j * 128:(j + 1) * 128])

                # Q projection + rope -> qT4
                for hp in range(2):
                    psq = pspool.tile([128, 512], F32, name="psq", tag="pj",
                                      bufs=2)
                    for k in range(16):
                        nc.tensor.matmul(
                            psq[:, :],
                            lhsT=wq_sb[hp][:, k * 128:(k + 1) * 128],
                            rhs=xt[k][:, :],
                            start=(k == 0), stop=(k == 15))
                    qraw = tpool.tile([128, 512], BF, name="qraw", tag="qraw")
                    nc.vector.tensor_copy(out=qraw[:, :], in_=psq[:, :])
                    rotq = tpool.tile([128, 512], BF, name="rotq", tag="rotq")
                    for i, (db, sb) in enumerate(
                            ((0, 32), (32, 0), (64, 96), (96, 64))):
                        eng = nc.scalar if (i + hp) % 2 == 0 else nc.gpsimd
                        eng.dma_start(out=rotq[db:db + 32, :],
                                      in_=qraw[sb:sb + 32, :])
                    q2 = tpool.tile([128, 512], BF, name="q2", tag="q2")
                    nc.vector.tensor_mul(q2[:, :], qraw[:, :],
                                         crep[:, cs:cs + 512])
                    nc.vector.tensor_mul(rotq[:, :], rotq[:, :],
                                         ssig[:, cs:cs + 512])
                    for ph in range(2):
                        h = 2 * hp + ph
                        dst = qv[:, b * 16 + 4 * g:b * 16 + 4 * g + 4, h, :]
                        s2 = q2[ph * 64:(ph + 1) * 64, :].rearrange(
                            "p (j r) -> p j r", r=128)
                        s3 = rotq[ph * 64:(ph + 1) * 64, :].rearrange(
                            "p (j r) -> p j r", r=128)
                        nc.vector.tensor_add(dst, s2, s3)

            def emit_pv(b, qb0, kb, scs, pvt):
                vbase = (b * 16 + kb) * 128
                for (pair, pr) in scs:
                    for jj, qb in enumerate(pair):
                        j = qb - qb0
                        nc.tensor.matmul(
                            pvt[j][0:VROW, :],
                            lhsT=vOnes[:, vbase:vbase + VROW],
                            rhs=pr[:, jj * 512:(jj + 1) * 512],
                            start=(kb == 0), stop=(kb == qb))
                        if kb == qb:
                            emit_norm(b, qb, pvt[j])

            def emit_norm(b, qb, pv):
                rsum = apool.tile([1, 512], F32, name="rsum", tag="rsum",
                                  bufs=2)
                nc.vector.tensor_copy(out=rsum[0:1, :], in_=pv[64:65, :])
                rsb = apool.tile([128, 512], F32, name="rsb", tag="rsb",
                                 bufs=2)
                nc.gpsimd.partition_broadcast(rsb[:, :], rsum[0:1, :])
                rep = apool.tile([128, 512], F32, name="rep", tag="rep",
                                 bufs=2)
                nc.vector.reciprocal_approx_fast(out=rep[:, :], in_=rsb[:, :])
                for t in range(2):
                    for ph in range(2):
                        h = 2 * t + ph
                        nc.vector.tensor_mul(
                            attnT[t][ph * 64:(ph + 1) * 64,
                                     b * S + qb * 128:b * S + (qb + 1) * 128],
                            pv[0:64, h * 128:(h + 1) * 128],
                            rep[ph * 64:(ph + 1) * 64, h * 128:(h + 1) * 128])

            def attn_group(b, m):
                qb0 = 2 * m
                pvt = [pspool.tile([VROW, 512], F32, name=f"pv{j}",
                                   tag=f"pv{j}") for j in range(2)]
                pend = []
                for kb in range(qb0 + 2):
                    active = [qb for qb in (qb0, qb0 + 1) if qb >= kb]
                    ksl = kT[0:64, b * S + kb * 128:b * S + (kb + 1) * 128]
                    sct = pspool.tile([128, 1024], F32, name="sc",
                                      tag="sc", bufs=2)
                    for jj, qb in enumerate(active):
                        qs = (b * 16 + qb) * 512
                        nc.tensor.matmul(
                            sct[:, jj * 512:(jj + 1) * 512],
                            lhsT=ksl, rhs=qT4[0:64, qs:qs + 512],
                            start=True, stop=True)
                    w = 512 * len(active)
                    pr = apool.tile([128, 1024], BF, name="probs",
                                    tag="probs", bufs=6)
                    nc.scalar.activation(out=pr[:, 0:w], in_=sct[:, 0:w],
                                         func=EXP, scale=0.125)
                    if active[0] == kb:
                        nc.vector.tensor_mul(pr[:, 0:512], pr[:, 0:512],
                                             dmask[:, :])
                    pend.append((kb, [(active, pr)]))
                    if len(pend) > 2:
                        pk, ps_ = pend.pop(0)
                        emit_pv(b, qb0, pk, ps_, pvt)
                for (pk, ps_) in pend:
                    emit_pv(b, qb0, pk, ps_, pvt)

            for b in range(B):
                for g in range(4):
                    proj_chunk(b * 4 + g)
                    for m in range(2 * g, 2 * g + 2):
                        attn_group(b, m)

            if dbg:
                nc.sync.dma_start(out=kT_dbg[:, :], in_=kT[0:64, :])
                nc.sync.dma_start(out=vO_dbg[:, :], in_=vOnes[:, :])
                nc.sync.dma_start(out=qT_dbg[:, :], in_=qT4[0:64, :])
                for t in range(2):
                    nc.sync.dma_start(out=aT_dbg[t, :, :], in_=attnT[t][:, :])

            # ---- output projection (partial: this core's 4 heads) ----
            # alternate wide (sc-tag, 2 banks) and narrow (pv-tag) psum groups
            # so 6 accumulation groups stay in flight.
            for rb in range(32):
                if rb % 2 == 0:
                    po = [pspool.tile([128, 1024], F32, name=f"po{dh}",
                                      tag="sc", bufs=2) for dh in range(2)]
                    tgt = [(po[0][:, 0:512], 0), (po[0][:, 512:1024], 1),
                           (po[1][:, 0:512], 2), (po[1][:, 512:1024], 3)]
                else:
                    ntags = ("pv0", "pv1", "pj", "pj")
                    po = [pspool.tile([128, 512], F32, name=f"po{dc}",
                                      tag=ntags[dc], bufs=2 if dc >= 2 else 1)
                          for dc in range(4)]
                    tgt = [(po[dc][:, :], dc) for dc in range(4)]
                for t in range(2):
                    for (dst, dc) in tgt:
                        nc.tensor.matmul(
                            dst,
                            lhsT=attnT[t][:, rb * 128:(rb + 1) * 128],
                            rhs=wot[t][:, dc * 512:(dc + 1) * 512],
                            start=(t == 0), stop=(t == 1))
                if rb % 2 == 0:
                    for dh in range(2):
                        ob = tpool.tile([128, 1024], BF, name="ob", tag="ob",
                                        bufs=4)
                        if dh % 2 == 0:
                            nc.vector.tensor_copy(out=ob[:, :],
                                                  in_=po[dh][:, :])
                        else:
                            nc.scalar.copy(out=ob[:, :], in_=po[dh][:, :])
                        eng = nc.sync if dh % 2 == 0 else nc.gpsimd
                        eng.dma_start(
                            out=out_d[rb * 128:(rb + 1) * 128,
                                      dh * 1024:(dh + 1) * 1024],
                            in_=ob[:, :])
                else:
                    for dc in range(4):
                        ob = tpool.tile([128, 512], BF, name="ob2", tag="ob2",
                                        bufs=4)
                        if dc % 2 == 0:
                            nc.vector.tensor_copy(out=ob[:, :],
                                                  in_=po[dc][:, :])
                        else:
                            nc.scalar.copy(out=ob[:, :], in_=po[dc][:, :])
                        eng = nc.sync if dc % 2 == 0 else nc.gpsimd
                        eng.dma_start(
                            out=out_d[rb * 128:(rb + 1) * 128,
                                      dc * 512:(dc + 1) * 512],
                            in_=ob[:, :])

    nc.compile()
    return nc


# --------------------------------------------------------------------------
# host-side sharding / layout prep
# --------------------------------------------------------------------------

def _deint(h):
    """de-interleaved column indices for head h (64 cols: evens then odds)."""
    return h * HD + np.concatenate([np.arange(0, HD, 2), np.arange(1, HD, 2)])


def _prep_shared(x, freqs_cos, freqs_sin, mask):
    xT = np.ascontiguousarray(
        x.reshape(BS, D).T).astype(BF16)                   # [D, BS]
    j = np.arange(128) % 32
    crep1 = freqs_cos[:, j].T                              # [128, S]
    crep = np.tile(crep1, (1, B)).astype(BF16)             # [128, BS]
    sgn = np.where((np.arange(128) // 32) % 2 == 0, -1.0, 1.0).astype(
        np.float32)
    ssig1 = freqs_sin[:, j].T * sgn[:, None]
    ssig = np.tile(ssig1, (1, B)).astype(BF16)
    dm = np.exp(mask[0:128, 0:128]).T                      # [128k, 128q]
    dmask = np.tile(dm, (1, 4)).astype(BF16)               # [128, 512]
    return xT, crep, ssig, dmask


def _prep_core(c, wq, wk, wv, wo):
    heads = [4 * c + h for h in range(4)]
    # wq: [2, 128, 2048] (head-pair, kdim-within-tile, k-tile*128 + head col)
    qcols = np.concatenate([_deint(h) for h in heads])     # [256]
    wq_c = wq[:, qcols].reshape(16, 128, 2, 128).transpose(2, 1, 0, 3)
    wq_c = np.ascontiguousarray(wq_c.reshape(2, 128, 2048)).astype(BF16)
    # wkv: [128, 2048] = per k-tile 128 cols: [wk deint 64 | wv natural 64]
    kcols = _deint(c)
    vcols = c * HD + np.arange(HD)
    wkv = np.concatenate([wk[:, kcols], wv[:, vcols]], axis=1)  # [D, 128]
    wkv_c = wkv.reshape(16, 128, 128).transpose(1, 0, 2)
    wkv_c = np.ascontiguousarray(wkv_c.reshape(128, 2048)).astype(BF16)
    # wo: [2, 128, D] rows = heads 2t, 2t+1 natural hd
    worows = np.concatenate([h * HD + np.arange(HD) for h in heads])
    wo_c = np.ascontiguousarray(wo[worows, :].reshape(2, 128, D)).astype(BF16)
    return wq_c, wkv_c, wo_c


def _assemble(results):
    acc = np.zeros((BS, D), np.float32)
    for i in range(NCORES):
        acc += np.asarray(results[i]["out"], dtype=np.float32)
    return np.ascontiguousarray(acc.reshape(B, S, D))


LAST_RUN_INFO = {}


def kernel(x, freqs_cos, freqs_sin, mask, wq, wk, wv, wo, start_pos=0):
    from concourse.bass_utils import run_bass_kernel_spmd

    x = np.asarray(x, dtype=np.float32)
    freqs_cos = np.asarray(freqs_cos, dtype=np.float32)
    freqs_sin = np.asarray(freqs_sin, dtype=np.float32)
    mask = np.asarray(mask, dtype=np.float32)
    wq = np.asarray(wq, dtype=np.float32)
    wk = np.asarray(wk, dtype=np.float32)
    wv = np.asarray(wv, dtype=np.float32)
    wo = np.asarray(wo, dtype=np.float32)

    xT, crep, ssig, dmask = _prep_shared(x, freqs_cos, freqs_sin, mask)
    in_maps = []
    for c in range(NCORES):
        wq_c, wkv_c, wo_c = _prep_core(c, wq, wk, wv, wo)
        in_maps.append({
            "xT": xT, "wq": wq_c, "wkv": wkv_c, "wo": wo_c,
            "crep": crep, "ssig": ssig, "dmask": dmask,
        })

    nc = _build_nc()

    trace = bool(int(os.environ.get("KERNEL_TRACE", "0")))
    kwargs = {}
    if trace:
        _install_ntff_hook()
        import concourse.bass_utils as bass_utils
        bass_utils.upload_artifacts = lambda tmpdir: tmpdir
        import tempfile
        tmpdir = tempfile.mkdtemp(prefix="attn_trace_")
        kwargs = {"trace": True, "tmpdir": tmpdir}

    res = run_bass_kernel_spmd(nc, in_maps, core_ids=list(range(NCORES)),
                               **kwargs)
    LAST_RUN_INFO.clear()
    LAST_RUN_INFO.update({
        "exec_time_ns": res.exec_time_ns,
        "tmpdir": kwargs.get("tmpdir"),
        "res": res,
    })
    return _assemble(res.results)


def _install_ntff_hook():
    if "antenv.axon_hooks" not in sys.modules:
        import antenv

        mod = types.ModuleType("antenv.axon_hooks")
        mod._hook = None
        mod.set_axon_ntff_profile_hook = lambda h: setattr(mod, "_hook", h)
        mod.get_axon_ntff_profile_hook = lambda: mod._hook
        sys.modules["antenv.axon_hooks"] = mod
        antenv.axon_hooks = mod
    from trn_agent_boot.trn_boot import _ntff_profile_via_ctypes
    from antenv.axon_hooks import set_axon_ntff_profile_hook as _set

    _set(_ntff_profile_via_ctypes("/opt/axon/libaxon_pjrt.so"))


# revision 18
# speedup vs baseline: 1.3284x; 1.0173x over previous
"""Distributed GQA attention kernel for 8 TRN2 NeuronCores.

Problem: B=2, S=2048, D=2048, 32 q-heads / 8 kv-heads, hd=64, causal + RoPE.

Strategy (kv-head tensor parallel, zero collectives):
  - Core c owns kv-head c (q-heads 4c..4c+3) for BOTH batches over ALL rows.
    Every core loads the full x (host-pretransposed to xT bf16) and projects
    Q (4 heads), K, V (1 kv head each) for all 4096 rows. K/V never leave the
    core, so there are NO collectives. Each core computes a PARTIAL output
    (its 4 heads x its 256 wo rows) and the host sums the 8 partials.
  - Because every core sees all rows, the causal structure is IDENTICAL on
    all cores (SPMD-compatible): per q-block qb only key blocks kb <= qb are
    computed -> ~47% of score/exp/PV work skipped exactly, with all matmuls
    staying N=512 wide (4 q-heads x 128 rows share one kv head -> one ksl
    stationary serves 4 heads; one vsl serves all q-blocks at a kb).
  - Attention runs fully "transposed": scoresT = ksl.T @ qT4 with keys on
    partitions; exp is merged into [128,1024] two-bank PSUM reads (one ACT
    instruction per 2 score tiles); only the diagonal tile gets a mask
    multiply. PV uses V in natural layout (via DMA-transpose from the
    projection) with a ones-column appended for the softmax denominator.
  - Softmax without max-subtraction: probs = exp(s/8); denominator from the
    ones-column; normalization applied to the PV output via
    reciprocal + partition_broadcast + elementwise multiply.
  - Matmuls in bf16; psums/softmax in fp32; partial output stored bf16.

kernel(**inputs) -> np.ndarray  takes full inputs, returns full [2,2048,2048].
"""

import functools
import os
import sys
import types

import numpy as np
import ml_dtypes

BF16 = ml_dtypes.bfloat16

B, S, D = 2, 2048, 2048
NH, NKV, HD = 32, 8, 64
BS = B * S               # 4096 rows total (b-major)
NB = S // 128            # 16 blocks per batch
NCORES = 8
VROW = HD + 1            # 65: [v | 1]


# --------------------------------------------------------------------------
# device graph (identical on all cores; per-core weights via input data)
# --------------------------------------------------------------------------

@functools.lru_cache(maxsize=None)
def _build_nc():
    import concourse.bacc as bacc
    import concourse.mybir as mybir
    import concourse.tile as tile

    BF = mybir.dt.bfloat16
    F32 = mybir.dt.float32
    EXP = mybir.ActivationFunctionType.Exp

    nc = bacc.Bacc(trn_type="TRN2", target_bir_lowering=False, debug=False,
                   num_devices=NCORES)

    dbg = bool(int(os.environ.get("KERNEL_DEBUG", "0")))
    if dbg:
        kT_dbg = nc.declare_dram_parameter("kT_dbg", [64, BS], BF,
                                           isOutput=True)
        vO_dbg = nc.declare_dram_parameter("vO_dbg", [128, 32 * 128], BF,
                                           isOutput=True)
        qT_dbg = nc.declare_dram_parameter("qT_dbg", [64, 32 * 512], BF,
                                           isOutput=True)
        aT_dbg = nc.declare_dram_parameter("aT_dbg", [2, 128, BS], BF,
                                           isOutput=True)

    xT_d = nc.declare_dram_parameter("xT", [D, BS], BF, isOutput=False)
    wq_d = nc.declare_dram_parameter("wq", [2, 128, 2048], BF, isOutput=False)
    wkv_d = nc.declare_dram_parameter("wkv", [128, 2048], BF, isOutput=False)
    wo_d = nc.declare_dram_parameter("wo", [2, 128, D], BF, isOutput=False)
    crep_d = nc.declare_dram_parameter("crep", [128, BS], BF, isOutput=False)
    ssig_d = nc.declare_dram_parameter("ssig", [128, BS], BF, isOutput=False)
    dmask_d = nc.declare_dram_parameter("dmask", [128, 512], BF, isOutput=False)
    out_d = nc.declare_dram_parameter("out", [BS, D], BF, isOutput=True)

    with tile.TileContext(nc) as tc:
        with tc.tile_pool(name="const", bufs=1) as cpool, \
             tc.tile_pool(name="persist", bufs=1) as ppool, \
             tc.tile_pool(name="xstream", bufs=27) as xpool, \
             tc.tile_pool(name="wstream", bufs=4) as wpool, \
             tc.tile_pool(name="work", bufs=3) as tpool, \
             tc.tile_pool(name="attn", bufs=3) as apool, \
             tc.tile_pool(name="ps", bufs=1, space="PSUM") as pspool:

            # ---- constants ----
            crep = cpool.tile([128, BS], BF, name="crep", tag="crep")
            nc.sync.dma_start(out=crep[:, :], in_=crep_d[:, :])
            ssig = cpool.tile([128, BS], BF, name="ssig", tag="ssig")
            nc.sync.dma_start(out=ssig[:, :], in_=ssig_d[:, :])
            dmask = cpool.tile([128, 512], BF, name="dmask", tag="dmask")
            nc.sync.dma_start(out=dmask[:, :], in_=dmask_d[:, :])
            wot = []
            for t in range(2):
                w_ = cpool.tile([128, D], BF, name=f"wot{t}", tag=f"wot{t}")
                nc.gpsimd.dma_start(out=w_[:, :], in_=wo_d[t, :, :])
                wot.append(w_)
            # resident projection weights (tiny: 2.5MB total)
            wkv_sb = cpool.tile([128, 2048], BF, name="wkv_sb", tag="wkv_sb")
            nc.sync.dma_start(out=wkv_sb[:, :], in_=wkv_d[:, :])
            wq_sb = []
            for hp in range(2):
                w_ = cpool.tile([128, 2048], BF, name=f"wq_sb{hp}",
                                tag=f"wq_sb{hp}")
                nc.sync.dma_start(out=w_[:, :], in_=wq_d[hp, :, :])
                wq_sb.append(w_)

            # ---- persistent activations ----
            kT = ppool.tile([64, BS], BF, name="kT", tag="kT")
            # V blocks padded to 128-col stride: dma_start_transpose needs
            # 128-byte-aligned output offsets (65-col stride corrupts).
            vOnes = ppool.tile([128, 32 * 128], BF, name="vOnes", tag="vOnes")
            vview = vOnes.rearrange("p (n w) -> p n w", w=128)
            nc.gpsimd.memset(vview[:, :, HD:HD + 1], 1.0)
            # qT4 cols: (b*16 + qb)*512 + h*128 + r   (h = head 0..3 local)
            qT4 = ppool.tile([64, 32 * 512], BF, name="qT4", tag="qT4")
            attnT = []
            for t in range(2):
                a_ = ppool.tile([128, BS], BF, name=f"attnT{t}", tag=f"attnT{t}")
                attnT.append(a_)

            # ---- interleaved projections + attention ----
            # Chunk c projects rows [c*512, (c+1)*512) (c = b*4 + g).
            # Attention runs in groups of 2 q-blocks (pv0/pv1 psum tags);
            # group (b, m) covers qb in {2m, 2m+1} and only needs chunks
            # <= b*4 + (2m+1)//4, so groups interleave with later chunks,
            # keeping PE dense through the whole kernel.
            qv = qT4.rearrange("p (blk h r) -> p blk h r", h=4, r=128)

            def proj_chunk(c):
                cs = c * 512
                b, g = divmod(c, 4)
                xt = []
                for k in range(16):
                    t_ = xpool.tile([128, 512], BF, name="xt", tag="xt")
                    eng = nc.sync if k % 2 == 0 else nc.gpsimd
                    eng.dma_start(out=t_[:, :],
                                  in_=xT_d[k * 128:(k + 1) * 128, cs:cs + 512])
                    xt.append(t_)

                # K|V fused projection: psum rows 0:64 = kT, 64:128 = vT
                ps = pspool.tile([128, 512], F32, name="pskv", tag="pj",
                                 bufs=2)
                for k in range(16):
                    nc.tensor.matmul(ps[:, :],
                                     lhsT=wkv_sb[:, k * 128:(k + 1) * 128],
                                     rhs=xt[k][:, :],
                                     start=(k == 0), stop=(k == 15))
                kvraw = tpool.tile([128, 512], BF, name="kvraw", tag="kvraw",
                                   bufs=6)
                nc.vector.tensor_copy(out=kvraw[:, :], in_=ps[:, :])
                # K rope (rows 0:64; [ev32|od32] de-interleaved)
                rot = tpool.tile([64, 512], BF, name="rot", tag="rot")
                nc.scalar.dma_start(out=rot[0:32, :], in_=kvraw[32:64, :])
                nc.scalar.dma_start(out=rot[32:64, :], in_=kvraw[0:32, :])
                nc.vector.tensor_mul(kT[0:64, cs:cs + 512], kvraw[0:64, :],
                                     crep[0:64, cs:cs + 512])
                nc.vector.tensor_mul(rot[0:64, :], rot[0:64, :],
                                     ssig[0:64, cs:cs + 512])
                nc.vector.tensor_add(kT[0:64, cs:cs + 512],
                                     kT[0:64, cs:cs + 512], rot[0:64, :])
                # V natural layout via DMA transpose (rows 64:128 of kvraw)
                for j in range(4):
                    blk = c * 4 + j
                    nc.scalar.dma_start_transpose(
                        out=vOnes[:, blk * 128:blk * 128 + HD],
                        in_=kvraw[64:128, j * 128:(j + 1) * 128])

                # Q projection + rope -> qT4
                for hp in range(2):
                    psq = pspool.tile([128, 512], F32, name="psq", tag="pj",
                                      bufs=2)
                    for k in range(16):
                        nc.tensor.matmul(
                            psq[:, :],
                            lhsT=wq_sb[hp][:, k * 128:(k + 1) * 128],
                            rhs=xt[k][:, :],
                            start=(k == 0), stop=(k == 15))
                    qraw = tpool.tile([128, 512], BF, name="qraw", tag="qraw")
                    nc.vector.tensor_copy(out=qraw[:, :], in_=psq[:, :])
                    rotq = tpool.tile([128, 512], BF, name="rotq", tag="rotq")
                    for i, (db, sb) in enumerate(
                            ((0, 32), (32, 0), (64, 96), (96, 64))):
                        eng = nc.scalar if (i + hp) % 2 == 0 else nc.gpsimd
                        eng.dma_start(out=rotq[db:db + 32, :],
                                      in_=qraw[sb:sb + 32, :])
                    q2 = tpool.tile([128, 512], BF, name="q2", tag="q2")
                    nc.vector.tensor_mul(q2[:, :], qraw[:, :],
                                         crep[:, cs:cs + 512])
                    nc.vector.tensor_mul(rotq[:, :], rotq[:, :],
                                         ssig[:, cs:cs + 512])
                    for ph in range(2):
                        h = 2 * hp + ph
                        dst = qv[:, b * 16 + 4 * g:b * 16 + 4 * g + 4, h, :]
                        s2 = q2[ph * 64:(ph + 1) * 64, :].rearrange(
                            "p (j r) -> p j r", r=128)
                        s3 = rotq[ph * 64:(ph + 1) * 64, :].rearrange(
                            "p (j r) -> p j r", r=128)
                        nc.vector.tensor_add(dst, s2, s3)

            def emit_pv(b, qb0, kb, scs, pvt):
                vbase = (b * 16 + kb) * 128
                for (pair, pr) in scs:
                    for jj, qb in enumerate(pair):
                        j = qb - qb0
                        nc.tensor.matmul(
                            pvt[j][0:VROW, :],
                            lhsT=vOnes[:, vbase:vbase + VROW],
                            rhs=pr[:, jj * 512:(jj + 1) * 512],
                            start=(kb == 0), stop=(kb == qb))
                        if kb == qb:
                            emit_norm(b, qb, pvt[j])

            def emit_norm(b, qb, pv):
                rsum = apool.tile([1, 512], F32, name="rsum", tag="rsum",
                                  bufs=2)
                nc.vector.tensor_copy(out=rsum[0:1, :], in_=pv[64:65, :])
                rsb = apool.tile([128, 512], F32, name="rsb", tag="rsb",
                                 bufs=2)
                nc.gpsimd.partition_broadcast(rsb[:, :], rsum[0:1, :])
                rep = apool.tile([128, 512], F32, name="rep", tag="rep",
                                 bufs=2)
                nc.vector.reciprocal_approx_fast(out=rep[:, :], in_=rsb[:, :])
                for t in range(2):
                    for ph in range(2):
                        h = 2 * t + ph
                        nc.vector.tensor_mul(
                            attnT[t][ph * 64:(ph + 1) * 64,
                                     b * S + qb * 128:b * S + (qb + 1) * 128],
                            pv[0:64, h * 128:(h + 1) * 128],
                            rep[ph * 64:(ph + 1) * 64, h * 128:(h + 1) * 128])

            def attn_group(b, m):
                qb0 = 2 * m
                pvt = [pspool.tile([VROW, 512], F32, name=f"pv{j}",
                                   tag=f"pv{j}") for j in range(2)]
                pend = []
                for kb in range(qb0 + 2):
                    active = [qb for qb in (qb0, qb0 + 1) if qb >= kb]
                    ksl = kT[0:64, b * S + kb * 128:b * S + (kb + 1) * 128]
                    sct = pspool.tile([128, 1024], F32, name="sc",
                                      tag="sc", bufs=2)
                    for jj, qb in enumerate(active):
                        qs = (b * 16 + qb) * 512
                        nc.tensor.matmul(
                            sct[:, jj * 512:(jj + 1) * 512],
                            lhsT=ksl, rhs=qT4[0:64, qs:qs + 512],
                            start=True, stop=True)
                    w = 512 * len(active)
                    pr = apool.tile([128, 1024], BF, name="probs",
                                    tag="probs", bufs=6)
                    nc.scalar.activation(out=pr[:, 0:w], in_=sct[:, 0:w],
                                         func=EXP, scale=0.125)
                    if active[0] == kb:
                        nc.vector.tensor_mul(pr[:, 0:512], pr[:, 0:512],
                                             dmask[:, :])
                    pend.append((kb, [(active, pr)]))
                    if len(pend) > 2:
                        pk, ps_ = pend.pop(0)
                        emit_pv(b, qb0, pk, ps_, pvt)
                for (pk, ps_) in pend:
                    emit_pv(b, qb0, pk, ps_, pvt)

            # proj chunks run one step ahead of the attention groups that
            # consume them, hiding the rope latency chain.
            sched = []
            for b in range(B):
                for g in range(4):
                    sched.append(("P", b * 4 + g))
            ai = []
            for b in range(B):
                for m in range(8):
                    ai.append((b, m))
            # interleave: P0 P1 [A0 A1] P2 [A2 A3] P3 ... (attention pair
            # after the chunk FOLLOWING its own chunk)
            order = []
            pi = 0
            order.append(sched[pi]); pi += 1
            for k in range(0, len(ai), 2):
                if pi < len(sched):
                    order.append(sched[pi]); pi += 1
                order.append(("A", ai[k]))
                order.append(("A", ai[k + 1]))
            for item in order:
                if item[0] == "P":
                    proj_chunk(item[1])
                else:
                    b_, m_ = item[1]
                    attn_group(b_, m_)

            if dbg:
                nc.sync.dma_start(out=kT_dbg[:, :], in_=kT[0:64, :])
                nc.sync.dma_start(out=vO_dbg[:, :], in_=vOnes[:, :])
                nc.sync.dma_start(out=qT_dbg[:, :], in_=qT4[0:64, :])
                for t in range(2):
                    nc.sync.dma_start(out=aT_dbg[t, :, :], in_=attnT[t][:, :])

            # ---- output projection (partial: this core's 4 heads) ----
            # alternate wide (sc-tag, 2 banks) and narrow (pv-tag) psum groups
            # so 6 accumulation groups stay in flight.
            for rb in range(32):
                if rb % 2 == 0:
                    po = [pspool.tile([128, 1024], F32, name=f"po{dh}",
                                      tag="sc", bufs=2) for dh in range(2)]
                    tgt = [(po[0][:, 0:512], 0), (po[0][:, 512:1024], 1),
                           (po[1][:, 0:512], 2), (po[1][:, 512:1024], 3)]
                else:
                    ntags = ("pv0", "pv1", "pj", "pj")
                    po = [pspool.tile([128, 512], F32, name=f"po{dc}",
                                      tag=ntags[dc], bufs=2 if dc >= 2 else 1)
                          for dc in range(4)]
                    tgt = [(po[dc][:, :], dc) for dc in range(4)]
                for t in range(2):
                    for (dst, dc) in tgt:
                        nc.tensor.matmul(
                            dst,
                            lhsT=attnT[t][:, rb * 128:(rb + 1) * 128],
                            rhs=wot[t][:, dc * 512:(dc + 1) * 512],
                            start=(t == 0), stop=(t == 1))
                if rb % 2 == 0:
                    for dh in range(2):
                        ob = tpool.tile([128, 1024], BF, name="ob", tag="ob",
                                        bufs=4)
                        if dh % 2 == 0:
                            nc.vector.tensor_copy(out=ob[:, :],
                                                  in_=po[dh][:, :])
                        else:
                            nc.scalar.copy(out=ob[:, :], in_=po[dh][:, :])
                        eng = nc.sync if dh % 2 == 0 else nc.gpsimd
                        eng.dma_start(
                            out=out_d[rb * 128:(rb + 1) * 128,
                                      dh * 1024:(dh + 1) * 1024],
                            in_=ob[:, :])
                else:
                    for dc in range(4):
                        ob = tpool.tile([128, 512], BF, name="ob2", tag="ob2",
                                        bufs=4)
                        if dc % 2 == 0:
                            nc.vector.tensor_copy(out=ob[:, :],
                                                  in_=po[dc][:, :])
                        else:
                            nc.scalar.copy(out=ob[:, :], in_=po[dc][:, :])
                        eng = nc.sync if dc % 2 == 0 else nc.gpsimd
                        eng.dma_start(
                            out=out_d[rb * 128:(rb + 1) * 128,
                                      dc * 512:(dc + 1) * 512],
                            in_=ob[:, :])

    nc.compile()
    return nc


# --------------------------------------------------------------------------
# host-side sharding / layout prep
# --------------------------------------------------------------------------

def _deint(h):
    """de-interleaved column indices for head h (64 cols: evens then odds)."""
    return h * HD + np.concatenate([np.arange(0, HD, 2), np.arange(1, HD, 2)])


def _prep_shared(x, freqs_cos, freqs_sin, mask):
    xT = np.ascontiguousarray(
        x.reshape(BS, D).T).astype(BF16)                   # [D, BS]
    j = np.arange(128) % 32
    crep1 = freqs_cos[:, j].T                              # [128, S]
    crep = np.tile(crep1, (1, B)).astype(BF16)             # [128, BS]
    sgn = np.where((np.arange(128) // 32) % 2 == 0, -1.0, 1.0).astype(
        np.float32)
    ssig1 = freqs_sin[:, j].T * sgn[:, None]
    ssig = np.tile(ssig1, (1, B)).astype(BF16)
    dm = np.exp(mask[0:128, 0:128]).T                      # [128k, 128q]
    dmask = np.tile(dm, (1, 4)).astype(BF16)               # [128, 512]
    return xT, crep, ssig, dmask


def _prep_core(c, wq, wk, wv, wo):
    heads = [4 * c + h for h in range(4)]
    # wq: [2, 128, 2048] (head-pair, kdim-within-tile, k-tile*128 + head col)
    qcols = np.concatenate([_deint(h) for h in heads])     # [256]
    wq_c = wq[:, qcols].reshape(16, 128, 2, 128).transpose(2, 1, 0, 3)
    wq_c = np.ascontiguousarray(wq_c.reshape(2, 128, 2048)).astype(BF16)
    # wkv: [128, 2048] = per k-tile 128 cols: [wk deint 64 | wv natural 64]
    kcols = _deint(c)
    vcols = c * HD + np.arange(HD)
    wkv = np.concatenate([wk[:, kcols], wv[:, vcols]], axis=1)  # [D, 128]
    wkv_c = wkv.reshape(16, 128, 128).transpose(1, 0, 2)
    wkv_c = np.ascontiguousarray(wkv_c.reshape(128, 2048)).astype(BF16)
    # wo: [2, 128, D] rows = heads 2t, 2t+1 natural hd
    worows = np.concatenate([h * HD + np.arange(HD) for h in heads])
    wo_c = np.ascontiguousarray(wo[worows, :].reshape(2, 128, D)).astype(BF16)
    return wq_c, wkv_c, wo_c


def _assemble(results):
    acc = np.zeros((BS, D), np.float32)
    for i in range(NCORES):
        acc += np.asarray(results[i]["out"], dtype=np.float32)
    return np.ascontiguousarray(acc.reshape(B, S, D))


LAST_RUN_INFO = {}


def kernel(x, freqs_cos, freqs_sin, mask, wq, wk, wv, wo, start_pos=0):
    from concourse.bass_utils import run_bass_kernel_spmd

    x = np.asarray(x, dtype=np.float32)
    freqs_cos = np.asarray(freqs_cos, dtype=np.float32)
    freqs_sin = np.asarray(freqs_sin, dtype=np.float32)
    mask = np.asarray(mask, dtype=np.float32)
    wq = np.asarray(wq, dtype=np.float32)
    wk = np.asarray(wk, dtype=np.float32)
    wv = np.asarray(wv, dtype=np.float32)
    wo = np.asarray(wo, dtype=np.float32)

    xT, crep, ssig, dmask = _prep_shared(x, freqs_cos, freqs_sin, mask)
    in_maps = []
    for c in range(NCORES):
        wq_c, wkv_c, wo_c = _prep_core(c, wq, wk, wv, wo)
        in_maps.append({
            "xT": xT, "wq": wq_c, "wkv": wkv_c, "wo": wo_c,
            "crep": crep, "ssig": ssig, "dmask": dmask,
        })

    nc = _build_nc()

    trace = bool(int(os.environ.get("KERNEL_TRACE", "0")))
    kwargs = {}
    if trace:
        _install_ntff_hook()
        import concourse.bass_utils as bass_utils
        bass_utils.upload_artifacts = lambda tmpdir: tmpdir
        import tempfile
        tmpdir = tempfile.mkdtemp(prefix="attn_trace_")
        kwargs = {"trace": True, "tmpdir": tmpdir}

    res = run_bass_kernel_spmd(nc, in_maps, core_ids=list(range(NCORES)),
                               **kwargs)
    LAST_RUN_INFO.clear()
    LAST_RUN_INFO.update({
        "exec_time_ns": res.exec_time_ns,
        "tmpdir": kwargs.get("tmpdir"),
        "res": res,
    })
    return _assemble(res.results)


def _install_ntff_hook():
    if "antenv.axon_hooks" not in sys.modules:
        import antenv

        mod = types.ModuleType("antenv.axon_hooks")
        mod._hook = None
        mod.set_axon_ntff_profile_hook = lambda h: setattr(mod, "_hook", h)
        mod.get_axon_ntff_profile_hook = lambda: mod._hook
        sys.modules["antenv.axon_hooks"] = mod
        antenv.axon_hooks = mod
    from trn_agent_boot.trn_boot import _ntff_profile_via_ctypes
    from antenv.axon_hooks import set_axon_ntff_profile_hook as _set

    _set(_ntff_profile_via_ctypes("/opt/axon/libaxon_pjrt.so"))


# revision 21
# speedup vs baseline: 1.4074x; 1.0595x over previous
"""Distributed GQA attention kernel for 8 TRN2 NeuronCores.

Problem: B=2, S=2048, D=2048, 32 q-heads / 8 kv-heads, hd=64, causal + RoPE.

Strategy (kv-head tensor parallel, zero collectives):
  - Core c owns kv-head c (q-heads 4c..4c+3) for BOTH batches over ALL rows.
    Every core loads the full x (host-pretransposed to xT bf16) and projects
    Q (4 heads), K, V (1 kv head each) for all 4096 rows. K/V never leave the
    core, so there are NO collectives. Each core computes a PARTIAL output
    (its 4 heads x its 256 wo rows) and the host sums the 8 partials.
  - Because every core sees all rows, the causal structure is IDENTICAL on
    all cores (SPMD-compatible): per q-block qb only key blocks kb <= qb are
    computed -> ~47% of score/exp/PV work skipped exactly, with all matmuls
    staying N=512 wide (4 q-heads x 128 rows share one kv head -> one ksl
    stationary serves 4 heads; one vsl serves all q-blocks at a kb).
  - Attention runs fully "transposed": scoresT = ksl.T @ qT4 with keys on
    partitions; exp is merged into [128,1024] two-bank PSUM reads (one ACT
    instruction per 2 score tiles); only the diagonal tile gets a mask
    multiply. PV uses V in natural layout (via DMA-transpose from the
    projection) with a ones-column appended for the softmax denominator.
  - Softmax without max-subtraction: probs = exp(s/8); denominator from the
    ones-column; normalization applied to the PV output via
    reciprocal + partition_broadcast + elementwise multiply.
  - Matmuls in bf16; psums/softmax in fp32; partial output stored bf16.

kernel(**inputs) -> np.ndarray  takes full inputs, returns full [2,2048,2048].
"""

import functools
import os
import sys
import types

import numpy as np
import ml_dtypes

BF16 = ml_dtypes.bfloat16

B, S, D = 2, 2048, 2048
NH, NKV, HD = 32, 8, 64
BS = B * S               # 4096 rows total (b-major)
NB = S // 128            # 16 blocks per batch
NCORES = 8
VROW = HD + 1            # 65: [v | 1]


# --------------------------------------------------------------------------
# device graph (identical on all cores; per-core weights via input data)
# --------------------------------------------------------------------------

@functools.lru_cache(maxsize=None)
def _build_nc():
    import concourse.bacc as bacc
    import concourse.mybir as mybir
    import concourse.tile as tile

    BF = mybir.dt.bfloat16
    F32 = mybir.dt.float32
    EXP = mybir.ActivationFunctionType.Exp

    nc = bacc.Bacc(trn_type="TRN2", target_bir_lowering=False, debug=False,
                   num_devices=NCORES)

    dbg = bool(int(os.environ.get("KERNEL_DEBUG", "0")))
    if dbg:
        kT_dbg = nc.declare_dram_parameter("kT_dbg", [64, BS], BF,
                                           isOutput=True)
        vO_dbg = nc.declare_dram_parameter("vO_dbg", [128, 32 * 128], BF,
                                           isOutput=True)
        qT_dbg = nc.declare_dram_parameter("qT_dbg", [64, 32 * 512], BF,
                                           isOutput=True)
        aT_dbg = nc.declare_dram_parameter("aT_dbg", [2, 128, BS], BF,
                                           isOutput=True)

    xT_d = nc.declare_dram_parameter("xT", [D, BS], BF, isOutput=False)
    wq_d = nc.declare_dram_parameter("wq", [2, 128, 2048], BF, isOutput=False)
    wkv_d = nc.declare_dram_parameter("wkv", [128, 2048], BF, isOutput=False)
    wo_d = nc.declare_dram_parameter("wo", [2, 128, D], BF, isOutput=False)
    crep_d = nc.declare_dram_parameter("crep", [128, BS], BF, isOutput=False)
    ssig_d = nc.declare_dram_parameter("ssig", [128, BS], BF, isOutput=False)
    dmask_d = nc.declare_dram_parameter("dmask", [128, 512], BF, isOutput=False)
    out_d = nc.declare_dram_parameter("out", [BS, D], BF, isOutput=True)

    with tile.TileContext(nc) as tc:
        with tc.tile_pool(name="const", bufs=1) as cpool, \
             tc.tile_pool(name="persist", bufs=1) as ppool, \
             tc.tile_pool(name="xstream", bufs=27) as xpool, \
             tc.tile_pool(name="wstream", bufs=4) as wpool, \
             tc.tile_pool(name="work", bufs=3) as tpool, \
             tc.tile_pool(name="attn", bufs=3) as apool, \
             tc.tile_pool(name="ps", bufs=1, space="PSUM") as pspool:

            # ---- constants ----
            # weights first: they gate the first projection chunk
            wkv_sb = cpool.tile([128, 2048], BF, name="wkv_sb", tag="wkv_sb")
            nc.sync.dma_start(out=wkv_sb[:, :], in_=wkv_d[:, :])
            wq_sb = []
            for hp in range(2):
                w_ = cpool.tile([128, 2048], BF, name=f"wq_sb{hp}",
                                tag=f"wq_sb{hp}")
                eng = nc.gpsimd if hp == 0 else nc.scalar
                eng.dma_start(out=w_[:, :], in_=wq_d[hp, :, :])
                wq_sb.append(w_)
            crep = cpool.tile([128, BS], BF, name="crep", tag="crep")
            nc.scalar.dma_start(out=crep[:, :], in_=crep_d[:, :])
            ssig = cpool.tile([128, BS], BF, name="ssig", tag="ssig")
            nc.scalar.dma_start(out=ssig[:, :], in_=ssig_d[:, :])
            dmask = cpool.tile([128, 512], BF, name="dmask", tag="dmask")
            nc.scalar.dma_start(out=dmask[:, :], in_=dmask_d[:, :])
            wot = []
            for t in range(2):
                w_ = cpool.tile([128, D], BF, name=f"wot{t}", tag=f"wot{t}")
                nc.gpsimd.dma_start(out=w_[:, :], in_=wo_d[t, :, :])
                wot.append(w_)

            # ---- persistent activations ----
            # rows 0:64 hold the data; rows 64:128 hold a DMA-duplicated copy
            # so K=64 score matmuls can row-pack two q-blocks per PE pass
            # (tile_position (0,0) and (64,0)). The upper half is free: SBUF
            # tiles always allocate 128 partitions.
            kT = ppool.tile([128, BS], BF, name="kT", tag="kT")
            # V blocks padded to 128-col stride: dma_start_transpose needs
            # 128-byte-aligned output offsets (65-col stride corrupts).
            vOnes = ppool.tile([128, 32 * 128], BF, name="vOnes", tag="vOnes")
            vview = vOnes.rearrange("p (n w) -> p n w", w=128)
            nc.gpsimd.memset(vview[:, :, HD:HD + 1], 1.0)
            # qT4 cols: (b*16 + qb)*512 + h*128 + r   (h = head 0..3 local)
            qT4 = ppool.tile([128, 32 * 512], BF, name="qT4", tag="qT4")
            attnT = []
            for t in range(2):
                a_ = ppool.tile([128, BS], BF, name=f"attnT{t}", tag=f"attnT{t}")
                attnT.append(a_)

            # ---- interleaved projections + attention ----
            # Chunk c projects rows [c*512, (c+1)*512) (c = b*4 + g).
            # Attention runs in groups of 2 q-blocks (pv0/pv1 psum tags);
            # group (b, m) covers qb in {2m, 2m+1} and only needs chunks
            # <= b*4 + (2m+1)//4, so groups interleave with later chunks,
            # keeping PE dense through the whole kernel.
            qv = qT4[0:64, :].rearrange("p (blk h r) -> p blk h r", h=4, r=128)

            def proj_chunk(c):
                cs = c * 512
                b, g = divmod(c, 4)
                xt = []
                for k in range(16):
                    t_ = xpool.tile([128, 512], BF, name="xt", tag="xt")
                    eng = nc.sync if k % 2 == 0 else nc.gpsimd
                    eng.dma_start(out=t_[:, :],
                                  in_=xT_d[k * 128:(k + 1) * 128, cs:cs + 512])
                    xt.append(t_)

                # K|V fused projection: psum rows 0:64 = kT, 64:128 = vT
                ps = pspool.tile([128, 512], F32, name="pskv", tag="pj",
                                 bufs=2)
                for k in range(16):
                    nc.tensor.matmul(ps[:, :],
                                     lhsT=wkv_sb[:, k * 128:(k + 1) * 128],
                                     rhs=xt[k][:, :],
                                     start=(k == 0), stop=(k == 15))
                kvraw = tpool.tile([128, 512], BF, name="kvraw", tag="kvraw",
                                   bufs=6)
                nc.vector.tensor_copy(out=kvraw[:, :], in_=ps[:, :])
                # K rope (rows 0:64; [ev32|od32] de-interleaved)
                rot = tpool.tile([64, 512], BF, name="rot", tag="rot")
                nc.scalar.dma_start(out=rot[0:32, :], in_=kvraw[32:64, :])
                nc.scalar.dma_start(out=rot[32:64, :], in_=kvraw[0:32, :])
                nc.vector.tensor_mul(kT[0:64, cs:cs + 512], kvraw[0:64, :],
                                     crep[0:64, cs:cs + 512])
                nc.vector.tensor_mul(rot[0:64, :], rot[0:64, :],
                                     ssig[0:64, cs:cs + 512])
                nc.vector.tensor_add(kT[0:64, cs:cs + 512],
                                     kT[0:64, cs:cs + 512], rot[0:64, :])
                nc.sync.dma_start(out=kT[64:128, cs:cs + 512],
                                  in_=kT[0:64, cs:cs + 512])
                # V natural layout via DMA transpose (rows 64:128 of kvraw)
                for j in range(4):
                    blk = c * 4 + j
                    nc.scalar.dma_start_transpose(
                        out=vOnes[:, blk * 128:blk * 128 + HD],
                        in_=kvraw[64:128, j * 128:(j + 1) * 128])

                # Q projection + rope -> qT4
                for hp in range(2):
                    psq = pspool.tile([128, 512], F32, name="psq", tag="pj",
                                      bufs=2)
                    for k in range(16):
                        nc.tensor.matmul(
                            psq[:, :],
                            lhsT=wq_sb[hp][:, k * 128:(k + 1) * 128],
                            rhs=xt[k][:, :],
                            start=(k == 0), stop=(k == 15))
                    qraw = tpool.tile([128, 512], BF, name="qraw", tag="qraw")
                    nc.vector.tensor_copy(out=qraw[:, :], in_=psq[:, :])
                    rotq = tpool.tile([128, 512], BF, name="rotq", tag="rotq")
                    for i, (db, sb) in enumerate(
                            ((0, 32), (32, 0), (64, 96), (96, 64))):
                        eng = nc.scalar if (i + hp) % 2 == 0 else nc.gpsimd
                        eng.dma_start(out=rotq[db:db + 32, :],
                                      in_=qraw[sb:sb + 32, :])
                    q2 = tpool.tile([128, 512], BF, name="q2", tag="q2")
                    nc.vector.tensor_mul(q2[:, :], qraw[:, :],
                                         crep[:, cs:cs + 512])
                    nc.vector.tensor_mul(rotq[:, :], rotq[:, :],
                                         ssig[:, cs:cs + 512])
                    for ph in range(2):
                        h = 2 * hp + ph
                        dst = qv[:, b * 16 + 4 * g:b * 16 + 4 * g + 4, h, :]
                        s2 = q2[ph * 64:(ph + 1) * 64, :].rearrange(
                            "p (j r) -> p j r", r=128)
                        s3 = rotq[ph * 64:(ph + 1) * 64, :].rearrange(
                            "p (j r) -> p j r", r=128)
                        nc.vector.tensor_add(dst, s2, s3)
                qc0 = (b * 16 + 4 * g) * 512
                nc.gpsimd.dma_start(out=qT4[64:128, qc0:qc0 + 2048],
                                    in_=qT4[0:64, qc0:qc0 + 2048])

            def emit_pv(b, qb0, kb, scs, pvt):
                vbase = (b * 16 + kb) * 128
                for (pair, pr) in scs:
                    for jj, qb in enumerate(pair):
                        j = qb - qb0
                        nc.tensor.matmul(
                            pvt[j][0:VROW, :],
                            lhsT=vOnes[:, vbase:vbase + VROW],
                            rhs=pr[:, jj * 512:(jj + 1) * 512],
                            start=(kb == 0), stop=(kb == qb))
                        if kb == qb:
                            emit_norm(b, qb, pvt[j])

            def emit_norm(b, qb, pv):
                rsum = apool.tile([1, 512], F32, name="rsum", tag="rsum",
                                  bufs=2)
                nc.vector.tensor_copy(out=rsum[0:1, :], in_=pv[64:65, :])
                rsb = apool.tile([128, 512], F32, name="rsb", tag="rsb",
                                 bufs=2)
                nc.gpsimd.partition_broadcast(rsb[:, :], rsum[0:1, :])
                rep = apool.tile([128, 512], F32, name="rep", tag="rep",
                                 bufs=2)
                nc.vector.reciprocal_approx_fast(out=rep[:, :], in_=rsb[:, :])
                for t in range(2):
                    for ph in range(2):
                        h = 2 * t + ph
                        nc.vector.tensor_mul(
                            attnT[t][ph * 64:(ph + 1) * 64,
                                     b * S + qb * 128:b * S + (qb + 1) * 128],
                            pv[0:64, h * 128:(h + 1) * 128],
                            rep[ph * 64:(ph + 1) * 64, h * 128:(h + 1) * 128])

            def attn_group(b, m):
                qb0 = 2 * m
                pvt = [pspool.tile([VROW, 512], F32, name=f"pv{j}",
                                   tag=f"pv{j}") for j in range(2)]
                pend = []
                for kb in range(qb0 + 2):
                    active = [qb for qb in (qb0, qb0 + 1) if qb >= kb]
                    ks = b * S + kb * 128
                    sct = pspool.tile([128, 1024], F32, name="sc",
                                      tag="sc", bufs=2)
                    for jj, qb in enumerate(active):
                        qs = (b * 16 + qb) * 512
                        lo = jj == 0
                        nc.tensor.matmul(
                            sct[:, jj * 512:(jj + 1) * 512],
                            lhsT=kT[0:64, ks:ks + 128] if lo
                            else kT[64:128, ks:ks + 128],
                            rhs=qT4[0:64, qs:qs + 512] if lo
                            else qT4[64:128, qs:qs + 512],
                            start=True, stop=True,
                            tile_position=(0, 0) if lo else (64, 0))
                    w = 512 * len(active)
                    pr = apool.tile([128, 1024], BF, name="probs",
                                    tag="probs", bufs=6)
                    nc.scalar.activation(out=pr[:, 0:w], in_=sct[:, 0:w],
                                         func=EXP, scale=0.125)
                    if active[0] == kb:
                        nc.vector.tensor_mul(pr[:, 0:512], pr[:, 0:512],
                                             dmask[:, :])
                    pend.append((kb, [(active, pr)]))
                    if len(pend) > 2:
                        pk, ps_ = pend.pop(0)
                        emit_pv(b, qb0, pk, ps_, pvt)
                for (pk, ps_) in pend:
                    emit_pv(b, qb0, pk, ps_, pvt)

            # proj chunks run one step ahead of the attention groups that
            # consume them, hiding the rope latency chain.
            sched = []
            for b in range(B):
                for g in range(4):
                    sched.append(("P", b * 4 + g))
            ai = []
            for b in range(B):
                for m in range(8):
                    ai.append((b, m))
            # interleave: P0 P1 [A0 A1] P2 [A2 A3] P3 ... (attention pair
            # after the chunk FOLLOWING its own chunk)
            order = []
            pi = 0
            order.append(sched[pi]); pi += 1
            for k in range(0, len(ai), 2):
                if pi < len(sched):
                    order.append(sched[pi]); pi += 1
                order.append(("A", ai[k]))
                order.append(("A", ai[k + 1]))
            for item in order:
                if item[0] == "P":
                    proj_chunk(item[1])
                else:
                    b_, m_ = item[1]
                    attn_group(b_, m_)

            if dbg:
                nc.sync.dma_start(out=kT_dbg[:, :], in_=kT[0:64, :])
                nc.sync.dma_start(out=vO_dbg[:, :], in_=vOnes[:, :])
                nc.sync.dma_start(out=qT_dbg[:, :], in_=qT4[0:64, :])
                for t in range(2):
                    nc.sync.dma_start(out=aT_dbg[t, :, :], in_=attnT[t][:, :])

            # ---- output projection (partial: this core's 4 heads) ----
            # alternate wide (sc-tag, 2 banks) and narrow (pv-tag) psum groups
            # so 6 accumulation groups stay in flight.
            for rb in range(32):
                if rb % 2 == 0:
                    po = [pspool.tile([128, 1024], F32, name=f"po{dh}",
                                      tag="sc", bufs=2) for dh in range(2)]
                    tgt = [(po[0][:, 0:512], 0), (po[0][:, 512:1024], 1),
                           (po[1][:, 0:512], 2), (po[1][:, 512:1024], 3)]
                else:
                    ntags = ("pv0", "pv1", "pj", "pj")
                    po = [pspool.tile([128, 512], F32, name=f"po{dc}",
                                      tag=ntags[dc], bufs=2 if dc >= 2 else 1)
                          for dc in range(4)]
                    tgt = [(po[dc][:, :], dc) for dc in range(4)]
                for t in range(2):
                    for (dst, dc) in tgt:
                        nc.tensor.matmul(
                            dst,
                            lhsT=attnT[t][:, rb * 128:(rb + 1) * 128],
                            rhs=wot[t][:, dc * 512:(dc + 1) * 512],
                            start=(t == 0), stop=(t == 1))
                if rb % 2 == 0:
                    for dh in range(2):
                        ob = tpool.tile([128, 1024], BF, name="ob", tag="ob",
                                        bufs=4)
                        if dh % 2 == 0:
                            nc.vector.tensor_copy(out=ob[:, :],
                                                  in_=po[dh][:, :])
                        else:
                            nc.scalar.copy(out=ob[:, :], in_=po[dh][:, :])
                        eng = nc.sync if dh % 2 == 0 else nc.gpsimd
                        eng.dma_start(
                            out=out_d[rb * 128:(rb + 1) * 128,
                                      dh * 1024:(dh + 1) * 1024],
                            in_=ob[:, :])
                else:
                    for dc in range(4):
                        ob = tpool.tile([128, 512], BF, name="ob2", tag="ob2",
                                        bufs=4)
                        if dc % 2 == 0:
                            nc.vector.tensor_copy(out=ob[:, :],
                                                  in_=po[dc][:, :])
                        else:
                            nc.scalar.copy(out=ob[:, :], in_=po[dc][:, :])
                        eng = nc.sync if dc % 2 == 0 else nc.gpsimd
                        eng.dma_start(
                            out=out_d[rb * 128:(rb + 1) * 128,
                                      dc * 512:(dc + 1) * 512],
                            in_=ob[:, :])

    nc.compile()
    return nc


# --------------------------------------------------------------------------
# host-side sharding / layout prep
# --------------------------------------------------------------------------

def _deint(h):
    """de-interleaved column indices for head h (64 cols: evens then odds)."""
    return h * HD + np.concatenate([np.arange(0, HD, 2), np.arange(1, HD, 2)])


def _prep_shared(x, freqs_cos, freqs_sin, mask):
    xT = np.ascontiguousarray(
        x.reshape(BS, D).T).astype(BF16)                   # [D, BS]
    j = np.arange(128) % 32
    crep1 = freqs_cos[:, j].T                              # [128, S]
    crep = np.tile(crep1, (1, B)).astype(BF16)             # [128, BS]
    sgn = np.where((np.arange(128) // 32) % 2 == 0, -1.0, 1.0).astype(
        np.float32)
    ssig1 = freqs_sin[:, j].T * sgn[:, None]
    ssig = np.tile(ssig1, (1, B)).astype(BF16)
    dm = np.exp(mask[0:128, 0:128]).T                      # [128k, 128q]
    dmask = np.tile(dm, (1, 4)).astype(BF16)               # [128, 512]
    return xT, crep, ssig, dmask


def _prep_core(c, wq, wk, wv, wo):
    heads = [4 * c + h for h in range(4)]
    # wq: [2, 128, 2048] (head-pair, kdim-within-tile, k-tile*128 + head col)
    qcols = np.concatenate([_deint(h) for h in heads])     # [256]
    wq_c = wq[:, qcols].reshape(16, 128, 2, 128).transpose(2, 1, 0, 3)
    wq_c = np.ascontiguousarray(wq_c.reshape(2, 128, 2048)).astype(BF16)
    # wkv: [128, 2048] = per k-tile 128 cols: [wk deint 64 | wv natural 64]
    kcols = _deint(c)
    vcols = c * HD + np.arange(HD)
    wkv = np.concatenate([wk[:, kcols], wv[:, vcols]], axis=1)  # [D, 128]
    wkv_c = wkv.reshape(16, 128, 128).transpose(1, 0, 2)
    wkv_c = np.ascontiguousarray(wkv_c.reshape(128, 2048)).astype(BF16)
    # wo: [2, 128, D] rows = heads 2t, 2t+1 natural hd
    worows = np.concatenate([h * HD + np.arange(HD) for h in heads])
    wo_c = np.ascontiguousarray(wo[worows, :].reshape(2, 128, D)).astype(BF16)
    return wq_c, wkv_c, wo_c


def _assemble(results):
    acc = np.zeros((BS, D), np.float32)
    for i in range(NCORES):
        acc += np.asarray(results[i]["out"], dtype=np.float32)
    return np.ascontiguousarray(acc.reshape(B, S, D))


LAST_RUN_INFO = {}


def kernel(x, freqs_cos, freqs_sin, mask, wq, wk, wv, wo, start_pos=0):
    from concourse.bass_utils import run_bass_kernel_spmd

    x = np.asarray(x, dtype=np.float32)
    freqs_cos = np.asarray(freqs_cos, dtype=np.float32)
    freqs_sin = np.asarray(freqs_sin, dtype=np.float32)
    mask = np.asarray(mask, dtype=np.float32)
    wq = np.asarray(wq, dtype=np.float32)
    wk = np.asarray(wk, dtype=np.float32)
    wv = np.asarray(wv, dtype=np.float32)
    wo = np.asarray(wo, dtype=np.float32)

    xT, crep, ssig, dmask = _prep_shared(x, freqs_cos, freqs_sin, mask)
    in_maps = []
    for c in range(NCORES):
        wq_c, wkv_c, wo_c = _prep_core(c, wq, wk, wv, wo)
        in_maps.append({
            "xT": xT, "wq": wq_c, "wkv": wkv_c, "wo": wo_c,
            "crep": crep, "ssig": ssig, "dmask": dmask,
        })

    nc = _build_nc()

    trace = bool(int(os.environ.get("KERNEL_TRACE", "0")))
    kwargs = {}
    if trace:
        _install_ntff_hook()
        import concourse.bass_utils as bass_utils
        bass_utils.upload_artifacts = lambda tmpdir: tmpdir
        import tempfile
        tmpdir = tempfile.mkdtemp(prefix="attn_trace_")
        kwargs = {"trace": True, "tmpdir": tmpdir}

    res = run_bass_kernel_spmd(nc, in_maps, core_ids=list(range(NCORES)),
                               **kwargs)
    LAST_RUN_INFO.clear()
    LAST_RUN_INFO.update({
        "exec_time_ns": res.exec_time_ns,
        "tmpdir": kwargs.get("tmpdir"),
        "res": res,
    })
    return _assemble(res.results)


def _install_ntff_hook():
    if "antenv.axon_hooks" not in sys.modules:
        import antenv

        mod = types.ModuleType("antenv.axon_hooks")
        mod._hook = None
        mod.set_axon_ntff_profile_hook = lambda h: setattr(mod, "_hook", h)
        mod.get_axon_ntff_profile_hook = lambda: mod._hook
        sys.modules["antenv.axon_hooks"] = mod
        antenv.axon_hooks = mod
    from trn_agent_boot.trn_boot import _ntff_profile_via_ctypes
    from antenv.axon_hooks import set_axon_ntff_profile_hook as _set

    _set(_ntff_profile_via_ctypes("/opt/axon/libaxon_pjrt.so"))


# revision 23
# speedup vs baseline: 1.5310x; 1.0879x over previous
"""Distributed GQA attention kernel for 8 TRN2 NeuronCores.

Problem: B=2, S=2048, D=2048, 32 q-heads / 8 kv-heads, hd=64, causal + RoPE.

Strategy (kv-head tensor parallel, zero collectives):
  - Core c owns kv-head c (q-heads 4c..4c+3) for BOTH batches over ALL rows.
    Every core loads the full x (host-pretransposed to xT bf16) and projects
    Q (4 heads), K, V (1 kv head each) for all 4096 rows. K/V never leave the
    core, so there are NO collectives. Each core computes a PARTIAL output
    (its 4 heads x its 256 wo rows) and the host sums the 8 partials.
  - Because every core sees all rows, the causal structure is IDENTICAL on
    all cores (SPMD-compatible): per q-block qb only key blocks kb <= qb are
    computed -> ~47% of score/exp/PV work skipped exactly, with all matmuls
    staying N=512 wide (4 q-heads x 128 rows share one kv head -> one ksl
    stationary serves 4 heads; one vsl serves all q-blocks at a kb).
  - Attention runs fully "transposed": scoresT = ksl.T @ qT4 with keys on
    partitions; exp is merged into [128,1024] two-bank PSUM reads (one ACT
    instruction per 2 score tiles); only the diagonal tile gets a mask
    multiply. PV uses V in natural layout (via DMA-transpose from the
    projection) with a ones-column appended for the softmax denominator.
  - Softmax without max-subtraction: probs = exp(s/8); denominator from the
    ones-column; normalization applied to the PV output via
    reciprocal + partition_broadcast + elementwise multiply.
  - Matmuls in bf16; psums/softmax in fp32; partial output stored bf16.

kernel(**inputs) -> np.ndarray  takes full inputs, returns full [2,2048,2048].
"""

import functools
import os
import sys
import types

import numpy as np
import ml_dtypes

BF16 = ml_dtypes.bfloat16

B, S, D = 2, 2048, 2048
NH, NKV, HD = 32, 8, 64
BS = B * S               # 4096 rows total (b-major)
NB = S // 128            # 16 blocks per batch
NCORES = 8
VROW = HD + 1            # 65: [v | 1]


# --------------------------------------------------------------------------
# device graph (identical on all cores; per-core weights via input data)
# --------------------------------------------------------------------------

@functools.lru_cache(maxsize=None)
def _build_nc():
    import concourse.bacc as bacc
    import concourse.mybir as mybir
    import concourse.tile as tile

    BF = mybir.dt.bfloat16
    F32 = mybir.dt.float32
    EXP = mybir.ActivationFunctionType.Exp

    nc = bacc.Bacc(trn_type="TRN2", target_bir_lowering=False, debug=False,
                   num_devices=NCORES)

    dbg = bool(int(os.environ.get("KERNEL_DEBUG", "0")))
    if dbg:
        kT_dbg = nc.declare_dram_parameter("kT_dbg", [64, BS], BF,
                                           isOutput=True)
        vO_dbg = nc.declare_dram_parameter("vO_dbg", [128, 32 * 128], BF,
                                           isOutput=True)
        qT_dbg = nc.declare_dram_parameter("qT_dbg", [64, 32 * 512], BF,
                                           isOutput=True)
        aT_dbg = nc.declare_dram_parameter("aT_dbg", [2, 128, BS], BF,
                                           isOutput=True)

    xT_d = nc.declare_dram_parameter("xT", [D, BS], BF, isOutput=False)
    wq_d = nc.declare_dram_parameter("wq", [2, 128, 2048], BF, isOutput=False)
    wkv_d = nc.declare_dram_parameter("wkv", [128, 2048], BF, isOutput=False)
    wo_d = nc.declare_dram_parameter("wo", [2, 128, D], BF, isOutput=False)
    crep_d = nc.declare_dram_parameter("crep", [128, BS], BF, isOutput=False)
    ssig_d = nc.declare_dram_parameter("ssig", [128, BS], BF, isOutput=False)
    dmask_d = nc.declare_dram_parameter("dmask", [128, 512], BF, isOutput=False)
    out_d = nc.declare_dram_parameter("out", [BS, D], BF, isOutput=True)

    with tile.TileContext(nc) as tc:
        with tc.tile_pool(name="const", bufs=1) as cpool, \
             tc.tile_pool(name="persist", bufs=1) as ppool, \
             tc.tile_pool(name="xstream", bufs=27) as xpool, \
             tc.tile_pool(name="wstream", bufs=4) as wpool, \
             tc.tile_pool(name="work", bufs=3) as tpool, \
             tc.tile_pool(name="attn", bufs=3) as apool, \
             tc.tile_pool(name="ps", bufs=1, space="PSUM") as pspool:

            # ---- constants ----
            # weights first: they gate the first projection chunk
            wkv_sb = cpool.tile([128, 2048], BF, name="wkv_sb", tag="wkv_sb")
            nc.sync.dma_start(out=wkv_sb[:, :], in_=wkv_d[:, :])
            wq_sb = []
            for hp in range(2):
                w_ = cpool.tile([128, 2048], BF, name=f"wq_sb{hp}",
                                tag=f"wq_sb{hp}")
                eng = nc.gpsimd if hp == 0 else nc.scalar
                eng.dma_start(out=w_[:, :], in_=wq_d[hp, :, :])
                wq_sb.append(w_)
            crep = cpool.tile([128, BS], BF, name="crep", tag="crep")
            nc.scalar.dma_start(out=crep[:, :], in_=crep_d[:, :])
            ssig = cpool.tile([128, BS], BF, name="ssig", tag="ssig")
            nc.scalar.dma_start(out=ssig[:, :], in_=ssig_d[:, :])
            dmask = cpool.tile([128, 512], BF, name="dmask", tag="dmask")
            nc.scalar.dma_start(out=dmask[:, :], in_=dmask_d[:, :])
            wot = []
            for t in range(2):
                w_ = cpool.tile([128, D], BF, name=f"wot{t}", tag=f"wot{t}")
                nc.gpsimd.dma_start(out=w_[:, :], in_=wo_d[t, :, :])
                wot.append(w_)

            # ---- persistent activations ----
            # rows 0:64 hold the data; rows 64:128 hold a DMA-duplicated copy
            # so K=64 score matmuls can row-pack two q-blocks per PE pass
            # (tile_position (0,0) and (64,0)). The upper half is free: SBUF
            # tiles always allocate 128 partitions.
            kT = ppool.tile([128, BS], BF, name="kT", tag="kT")
            # V blocks padded to 128-col stride: dma_start_transpose needs
            # 128-byte-aligned output offsets (65-col stride corrupts).
            vOnes = ppool.tile([128, 32 * 128], BF, name="vOnes", tag="vOnes")
            vview = vOnes.rearrange("p (n w) -> p n w", w=128)
            nc.gpsimd.memset(vview[:, :, HD:HD + 1], 1.0)
            # qT4 cols: (b*16 + qb)*512 + h*128 + r   (h = head 0..3 local)
            qT4 = ppool.tile([128, 32 * 512], BF, name="qT4", tag="qT4")
            attnT = []
            for t in range(2):
                a_ = ppool.tile([128, BS], BF, name=f"attnT{t}", tag=f"attnT{t}")
                attnT.append(a_)

            # ---- interleaved projections + attention ----
            # Chunk c projects rows [c*512, (c+1)*512) (c = b*4 + g).
            # Attention runs in groups of 2 q-blocks (pv0/pv1 psum tags);
            # group (b, m) covers qb in {2m, 2m+1} and only needs chunks
            # <= b*4 + (2m+1)//4, so groups interleave with later chunks,
            # keeping PE dense through the whole kernel.
            qv = qT4[0:64, :].rearrange("p (blk h r) -> p blk h r", h=4, r=128)

            def proj_chunk(c):
                cs = c * 512
                b, g = divmod(c, 4)
                xt = []
                for k in range(16):
                    t_ = xpool.tile([128, 512], BF, name="xt", tag="xt")
                    eng = nc.sync if k % 2 == 0 else nc.gpsimd
                    eng.dma_start(out=t_[:, :],
                                  in_=xT_d[k * 128:(k + 1) * 128, cs:cs + 512])
                    xt.append(t_)

                # K|V fused projection: psum rows 0:64 = kT, 64:128 = vT
                ps = pspool.tile([128, 512], F32, name="pskv", tag="pj",
                                 bufs=2)
                for k in range(16):
                    nc.tensor.matmul(ps[:, :],
                                     lhsT=wkv_sb[:, k * 128:(k + 1) * 128],
                                     rhs=xt[k][:, :],
                                     start=(k == 0), stop=(k == 15))
                kvraw = tpool.tile([128, 512], BF, name="kvraw", tag="kvraw",
                                   bufs=6)
                nc.vector.tensor_copy(out=kvraw[:, :], in_=ps[:, :])
                # K rope (rows 0:64; [ev32|od32] de-interleaved)
                rot = tpool.tile([64, 512], BF, name="rot", tag="rot")
                nc.sync.dma_start(out=rot[0:32, :], in_=kvraw[32:64, :])
                nc.sync.dma_start(out=rot[32:64, :], in_=kvraw[0:32, :])
                nc.vector.tensor_mul(kT[0:64, cs:cs + 512], kvraw[0:64, :],
                                     crep[0:64, cs:cs + 512])
                nc.vector.tensor_mul(rot[0:64, :], rot[0:64, :],
                                     ssig[0:64, cs:cs + 512])
                nc.vector.tensor_add(kT[0:64, cs:cs + 512],
                                     kT[0:64, cs:cs + 512], rot[0:64, :])
                nc.sync.dma_start(out=kT[64:128, cs:cs + 512],
                                  in_=kT[0:64, cs:cs + 512])
                # V natural layout via DMA transpose (rows 64:128 of kvraw)
                for j in range(4):
                    blk = c * 4 + j
                    nc.sync.dma_start_transpose(
                        out=vOnes[:, blk * 128:blk * 128 + HD],
                        in_=kvraw[64:128, j * 128:(j + 1) * 128])

                # Q projection + rope -> qT4
                for hp in range(2):
                    psq = pspool.tile([128, 512], F32, name="psq", tag="pj",
                                      bufs=2)
                    for k in range(16):
                        nc.tensor.matmul(
                            psq[:, :],
                            lhsT=wq_sb[hp][:, k * 128:(k + 1) * 128],
                            rhs=xt[k][:, :],
                            start=(k == 0), stop=(k == 15))
                    qraw = tpool.tile([128, 512], BF, name="qraw", tag="qraw")
                    nc.vector.tensor_copy(out=qraw[:, :], in_=psq[:, :])
                    rotq = tpool.tile([128, 512], BF, name="rotq", tag="rotq")
                    for i, (db, sb) in enumerate(
                            ((0, 32), (32, 0), (64, 96), (96, 64))):
                        eng = nc.scalar if (i + hp) % 2 == 0 else nc.gpsimd
                        eng.dma_start(out=rotq[db:db + 32, :],
                                      in_=qraw[sb:sb + 32, :])
                    q2 = tpool.tile([128, 512], BF, name="q2", tag="q2")
                    nc.vector.tensor_mul(q2[:, :], qraw[:, :],
                                         crep[:, cs:cs + 512])
                    nc.vector.tensor_mul(rotq[:, :], rotq[:, :],
                                         ssig[:, cs:cs + 512])
                    for ph in range(2):
                        h = 2 * hp + ph
                        dst = qv[:, b * 16 + 4 * g:b * 16 + 4 * g + 4, h, :]
                        s2 = q2[ph * 64:(ph + 1) * 64, :].rearrange(
                            "p (j r) -> p j r", r=128)
                        s3 = rotq[ph * 64:(ph + 1) * 64, :].rearrange(
                            "p (j r) -> p j r", r=128)
                        nc.vector.tensor_add(dst, s2, s3)
                qc0 = (b * 16 + 4 * g) * 512
                nc.gpsimd.dma_start(out=qT4[64:128, qc0:qc0 + 2048],
                                    in_=qT4[0:64, qc0:qc0 + 2048])

            def emit_pv(b, qb0, kb, scs, pvt):
                vbase = (b * 16 + kb) * 128
                for (pair, pr) in scs:
                    for jj, qb in enumerate(pair):
                        j = qb - qb0
                        nc.tensor.matmul(
                            pvt[j][0:VROW, :],
                            lhsT=vOnes[:, vbase:vbase + VROW],
                            rhs=pr[:, jj * 512:(jj + 1) * 512],
                            start=(kb == 0), stop=(kb == qb))
                        if kb == qb:
                            emit_norm(b, qb, pvt[j])

            def emit_norm(b, qb, pv):
                # read sums + copy pv rows to SBUF immediately: the psum bank
                # frees after these two reads instead of after the whole chain
                rsum = apool.tile([1, 512], F32, name="rsum", tag="rsum",
                                  bufs=2)
                nc.vector.tensor_copy(out=rsum[0:1, :], in_=pv[64:65, :])
                pvs = apool.tile([64, 512], F32, name="pvs", tag="pvs",
                                 bufs=2)
                nc.vector.tensor_copy(out=pvs[0:64, :], in_=pv[0:64, :])
                rsb = apool.tile([128, 512], F32, name="rsb", tag="rsb",
                                 bufs=2)
                nc.gpsimd.partition_broadcast(rsb[:, :], rsum[0:1, :])
                rep = apool.tile([128, 512], F32, name="rep", tag="rep",
                                 bufs=2)
                nc.vector.reciprocal_approx_fast(out=rep[0:64, :],
                                                 in_=rsb[0:64, :])
                # rep rows are a broadcast: base-0 slices keep both SBUF
                # inputs partition-aligned (walrus NCC_IBIR297)
                for t in range(2):
                    for ph in range(2):
                        h = 2 * t + ph
                        nc.vector.tensor_mul(
                            attnT[t][ph * 64:(ph + 1) * 64,
                                     b * S + qb * 128:b * S + (qb + 1) * 128],
                            pvs[0:64, h * 128:(h + 1) * 128],
                            rep[0:64, h * 128:(h + 1) * 128])

            def outproj_rbs(rbs):
                for i, rb in enumerate(rbs):
                    for dc in range(4):
                        po = pspool.tile([128, 512], F32, name="po",
                                         tag="pj", bufs=2)
                        for t in range(2):
                            nc.tensor.matmul(
                                po[:, :],
                                lhsT=attnT[t][:, rb * 128:(rb + 1) * 128],
                                rhs=wot[t][:, dc * 512:(dc + 1) * 512],
                                start=(t == 0), stop=(t == 1))
                        ob = tpool.tile([128, 512], BF, name="ob", tag="ob",
                                        bufs=6)
                        nc.vector.tensor_copy(out=ob[:, :], in_=po[:, :])
                        eng = nc.sync if (i + dc) % 2 == 0 else nc.gpsimd
                        eng.dma_start(
                            out=out_d[rb * 128:(rb + 1) * 128,
                                      dc * 512:(dc + 1) * 512],
                            in_=ob[:, :])

            def attn_group(b, m):
                qb0 = 2 * m
                pvt = [pspool.tile([VROW, 512], F32, name=f"pv{j}",
                                   tag=f"pv{j}") for j in range(2)]
                pend = []
                for kb in range(qb0 + 2):
                    active = [qb for qb in (qb0, qb0 + 1) if qb >= kb]
                    ks = b * S + kb * 128
                    sct = pspool.tile([128, 1024], F32, name="sc",
                                      tag="sc", bufs=2)
                    for jj, qb in enumerate(active):
                        qs = (b * 16 + qb) * 512
                        lo = jj == 0
                        nc.tensor.matmul(
                            sct[:, jj * 512:(jj + 1) * 512],
                            lhsT=kT[0:64, ks:ks + 128] if lo
                            else kT[64:128, ks:ks + 128],
                            rhs=qT4[0:64, qs:qs + 512] if lo
                            else qT4[64:128, qs:qs + 512],
                            start=True, stop=True,
                            tile_position=(0, 0) if lo else (64, 0))
                    w = 512 * len(active)
                    pr = apool.tile([128, 1024], BF, name="probs",
                                    tag="probs", bufs=6)
                    nc.scalar.activation(out=pr[:, 0:w], in_=sct[:, 0:w],
                                         func=EXP, scale=0.125)
                    if active[0] == kb:
                        nc.vector.tensor_mul(pr[:, 0:512], pr[:, 0:512],
                                             dmask[:, :])
                    pend.append((kb, [(active, pr)]))
                    if len(pend) > 2:
                        pk, ps_ = pend.pop(0)
                        emit_pv(b, qb0, pk, ps_, pvt)
                for (pk, ps_) in pend:
                    emit_pv(b, qb0, pk, ps_, pvt)

            # proj chunks run one step ahead of the attention groups that
            # consume them, hiding the rope latency chain.
            sched = []
            for b in range(B):
                for g in range(4):
                    sched.append(("P", b * 4 + g))
            ai = []
            for b in range(B):
                for m in range(8):
                    ai.append((b, m))
            # interleave: P0 P1 [A0 A1] P2 [A2 A3] P3 ... (attention pair
            # after the chunk FOLLOWING its own chunk)
            order = []
            pi = 0
            order.append(sched[pi]); pi += 1
            for k in range(0, len(ai), 2):
                if pi < len(sched):
                    order.append(sched[pi]); pi += 1
                order.append(("A", ai[k]))
                order.append(("A", ai[k + 1]))
            for item in order:
                if item[0] == "P":
                    proj_chunk(item[1])
                else:
                    b_, m_ = item[1]
                    attn_group(b_, m_)
                    if b_ == 1:
                        qb0 = 2 * m_
                        outproj_rbs([qb0, qb0 + 1, 16 + qb0, 17 + qb0])

            if dbg:
                nc.sync.dma_start(out=kT_dbg[:, :], in_=kT[0:64, :])
                nc.sync.dma_start(out=vO_dbg[:, :], in_=vOnes[:, :])
                nc.sync.dma_start(out=qT_dbg[:, :], in_=qT4[0:64, :])
                for t in range(2):
                    nc.sync.dma_start(out=aT_dbg[t, :, :], in_=attnT[t][:, :])

    nc.compile()
    return nc


# --------------------------------------------------------------------------
# host-side sharding / layout prep
# --------------------------------------------------------------------------

def _deint(h):
    """de-interleaved column indices for head h (64 cols: evens then odds)."""
    return h * HD + np.concatenate([np.arange(0, HD, 2), np.arange(1, HD, 2)])


def _prep_shared(x, freqs_cos, freqs_sin, mask):
    xT = np.ascontiguousarray(
        x.reshape(BS, D).T).astype(BF16)                   # [D, BS]
    j = np.arange(128) % 32
    crep1 = freqs_cos[:, j].T                              # [128, S]
    crep = np.tile(crep1, (1, B)).astype(BF16)             # [128, BS]
    sgn = np.where((np.arange(128) // 32) % 2 == 0, -1.0, 1.0).astype(
        np.float32)
    ssig1 = freqs_sin[:, j].T * sgn[:, None]
    ssig = np.tile(ssig1, (1, B)).astype(BF16)
    dm = np.exp(mask[0:128, 0:128]).T                      # [128k, 128q]
    dmask = np.tile(dm, (1, 4)).astype(BF16)               # [128, 512]
    return xT, crep, ssig, dmask


def _prep_core(c, wq, wk, wv, wo):
    heads = [4 * c + h for h in range(4)]
    # wq: [2, 128, 2048] (head-pair, kdim-within-tile, k-tile*128 + head col)
    qcols = np.concatenate([_deint(h) for h in heads])     # [256]
    wq_c = wq[:, qcols].reshape(16, 128, 2, 128).transpose(2, 1, 0, 3)
    wq_c = np.ascontiguousarray(wq_c.reshape(2, 128, 2048)).astype(BF16)
    # wkv: [128, 2048] = per k-tile 128 cols: [wk deint 64 | wv natural 64]
    kcols = _deint(c)
    vcols = c * HD + np.arange(HD)
    wkv = np.concatenate([wk[:, kcols], wv[:, vcols]], axis=1)  # [D, 128]
    wkv_c = wkv.reshape(16, 128, 128).transpose(1, 0, 2)
    wkv_c = np.ascontiguousarray(wkv_c.reshape(128, 2048)).astype(BF16)
    # wo: [2, 128, D] rows = heads 2t, 2t+1 natural hd
    worows = np.concatenate([h * HD + np.arange(HD) for h in heads])
    wo_c = np.ascontiguousarray(wo[worows, :].reshape(2, 128, D)).astype(BF16)
    return wq_c, wkv_c, wo_c


def _assemble(results):
    acc = np.zeros((BS, D), np.float32)
    for i in range(NCORES):
        acc += np.asarray(results[i]["out"], dtype=np.float32)
    return np.ascontiguousarray(acc.reshape(B, S, D))


LAST_RUN_INFO = {}


def kernel(x, freqs_cos, freqs_sin, mask, wq, wk, wv, wo, start_pos=0):
    from concourse.bass_utils import run_bass_kernel_spmd

    x = np.asarray(x, dtype=np.float32)
    freqs_cos = np.asarray(freqs_cos, dtype=np.float32)
    freqs_sin = np.asarray(freqs_sin, dtype=np.float32)
    mask = np.asarray(mask, dtype=np.float32)
    wq = np.asarray(wq, dtype=np.float32)
    wk = np.asarray(wk, dtype=np.float32)
    wv = np.asarray(wv, dtype=np.float32)
    wo = np.asarray(wo, dtype=np.float32)

    xT, crep, ssig, dmask = _prep_shared(x, freqs_cos, freqs_sin, mask)
    in_maps = []
    for c in range(NCORES):
        wq_c, wkv_c, wo_c = _prep_core(c, wq, wk, wv, wo)
        in_maps.append({
            "xT": xT, "wq": wq_c, "wkv": wkv_c, "wo": wo_c,
            "crep": crep, "ssig": ssig, "dmask": dmask,
        })

    nc = _build_nc()

    trace = bool(int(os.environ.get("KERNEL_TRACE", "0")))
    kwargs = {}
    if trace:
        _install_ntff_hook()
        import concourse.bass_utils as bass_utils
        bass_utils.upload_artifacts = lambda tmpdir: tmpdir
        import tempfile
        tmpdir = tempfile.mkdtemp(prefix="attn_trace_")
        kwargs = {"trace": True, "tmpdir": tmpdir}

    res = run_bass_kernel_spmd(nc, in_maps, core_ids=list(range(NCORES)),
                               **kwargs)
    LAST_RUN_INFO.clear()
    LAST_RUN_INFO.update({
        "exec_time_ns": res.exec_time_ns,
        "tmpdir": kwargs.get("tmpdir"),
        "res": res,
    })
    return _assemble(res.results)


def _install_ntff_hook():
    if "antenv.axon_hooks" not in sys.modules:
        import antenv

        mod = types.ModuleType("antenv.axon_hooks")
        mod._hook = None
        mod.set_axon_ntff_profile_hook = lambda h: setattr(mod, "_hook", h)
        mod.get_axon_ntff_profile_hook = lambda: mod._hook
        sys.modules["antenv.axon_hooks"] = mod
        antenv.axon_hooks = mod
    from trn_agent_boot.trn_boot import _ntff_profile_via_ctypes
    from antenv.axon_hooks import set_axon_ntff_profile_hook as _set

    _set(_ntff_profile_via_ctypes("/opt/axon/libaxon_pjrt.so"))
